# revision 16
# baseline (speedup 1.0000x reference)
"""ClusterAwareAttention Trainium2 kernel (8 NeuronCores, axon/PJRT path).

Sharding: data-parallel over (batch, sequence-half) -> 8 shards of 8192 rows.

Two launches:
  Pass 1: per-shard cluster pooling partial sums  xp = A_loc^T @ x_loc  (64, 256)
          in bf16 (p-major contiguous layout for full-rate DMA).
  Host:   reduce halves, build pooled K/V-derived constants:
            Wk_cl (x @ Wk_cl = q @ k_cluster^T * scale, folded through W_q),
            VBD block-diagonal v_cluster, cluster-bias, indicators; global
            pow2 scales Sk (fp8 Wk_cl) and Sv (fp8 v_cluster).
  Pass 2: per-shard fused attention, cluster-major (n on the free axis,
          512-col groups):
            fp8 DoubleRow logits (256-deep contraction in one stream)
            + bf16 cluster bias -> exp (fp8 out, scale=1/Sk)
            -> fp8 DoubleRow denominators -> DVE reciprocal (bf16)
            -> bf16 broadcast matmul -> fp8 DoubleRow attn@V
            -> DVE normalize (bf16) -> bf16 output projection
            -> direct PSUM->DRAM y writes (f32; bias + 1/Sv applied on host).

Precision (validated on host): pooling bf16, logits fp8 (scaled), P fp8,
v_cluster fp8 (scaled), everything else bf16/f32 accum. rel err ~8e-3.
"""

import json
import os
from functools import lru_cache

import numpy as np

import concourse.bass as bass
import concourse.tile as tile
from concourse import mybir
from concourse.bass_utils import run_bass_kernel_spmd

import ml_dtypes

BF16 = ml_dtypes.bfloat16
E4 = ml_dtypes.float8_e4m3   # mybir.dt.float8e4 <-> jnp/ml_dtypes float8_e4m3 (max 240)

B, N, C, H, K = 4, 16384, 256, 8, 64
D = C // H
EPS = 1e-8
SCALE = D ** -0.5
NLOC = N // 2           # rows per shard
F = 512                 # group size (n on the free axis)
NGROUPS = NLOC // F     # 16
MG = 2048               # DMA macro-group (4 groups per input DMA chunk)
NMG = NLOC // MG        # 4
NCORES = 8

f32 = mybir.dt.float32
bf16 = mybir.dt.bfloat16
fp8 = mybir.dt.float8e4
DR = mybir.MatmulPerfMode.DoubleRow


# --------------------------------------------------------------------------
# BIR fixup: this container's walrus rejects instructions with >1 sync wait.
# Split extra waits onto single-wait EventSemaphore instructions just before.
# --------------------------------------------------------------------------
def _split_block(bb, counter):
    insts = bb.get("instructions")
    if insts:
        new_insts = []
        for inst in insts:
            si = inst.get("sync_info") or {}
            waits = si.get("on_wait") or []
            if len(waits) > 1:
                for w in waits[:-1]:
                    counter[0] += 1
                    new_insts.append(
                        {
                            "debug": inst.get("debug", 0),
                            "engine": inst["engine"],
                            "ins": [],
                            "name": f"WSPLIT-{counter[0]}",
                            "opcode": "EventSemaphore",
                            "outs": [],
                            "sync_info": {"on_update": [], "on_wait": [w]},
                        }
                    )
                si = dict(si)
                si["on_wait"] = [waits[-1]]
                inst = dict(inst)
                inst["sync_info"] = si
            new_insts.append(inst)
        bb["instructions"] = new_insts
    for sub in bb.get("blocks", []) or []:
        _split_block(sub, counter)


def _fixup_bir_json(bir_json: bytes) -> bytes:
    bir = json.loads(bir_json)
    counter = [0]
    for fn in bir.get("functions", []):
        for bb in fn.get("blocks", []) or []:
            _split_block(bb, counter)
    return json.dumps(bir).encode()


LAST_EXEC_NS = None
TRACE_DIRS = []


def _scalar_recip(nc, out, in_):
    """Raw ACT Reciprocal (bass blocks the wrapper for accuracy reasons;
    our 1/s operands are well-conditioned and the rel-err gate validates)."""
    se = nc.scalar
    ins = [se.lower_ap(in_)]
    for v in (0.0, 1.0, 0.0):  # bias, scale, alpha
        ins.append(mybir.ImmediateValue(dtype=mybir.dt.float32, value=v))
    return se.add_instruction(
        mybir.InstActivation(
            name=nc.get_next_instruction_name(),
            func=mybir.ActivationFunctionType.Reciprocal,
            ins=ins,
            outs=[se.lower_ap(out)],
        )
    )


def _install_profhook():
    import sys
    import types

    if "antenv.axon_hooks" in sys.modules:
        return
    import antenv

    mod = types.ModuleType("antenv.axon_hooks")
    _hook = [None]
    mod.set_axon_ntff_profile_hook = lambda h: _hook.__setitem__(0, h)
    mod.get_axon_ntff_profile_hook = lambda: _hook[0]
    sys.modules["antenv.axon_hooks"] = mod
    antenv.axon_hooks = mod
    from trn_agent_boot.trn_boot import _ntff_profile_via_ctypes

    mod.set_axon_ntff_profile_hook(
        _ntff_profile_via_ctypes("/opt/axon/libaxon_pjrt.so")
    )


_fixup_installed = False


def _install_fixup():
    global _fixup_installed
    if _fixup_installed:
        return
    _fixup_installed = True
    import concourse.bass_utils as bu
    import concourse.bass2jax as b2j

    orig = bu.compile_bir_kernel

    def patched(bir_json, tmpdir, neff_name="file.neff"):
        return orig(_fixup_bir_json(bir_json), tmpdir, neff_name=neff_name)

    bu.compile_bir_kernel = patched
    b2j.compile_bir_kernel = patched


# --------------------------------------------------------------------------
# Pass 1: xp[k, c] = sum_n A_loc[n, k] * x_loc[n, c], bf16 in / f32 accum.
# p-major layout: row n = 64*p + i  ->  x[p, i, c] contiguous per partition.
# --------------------------------------------------------------------------
@lru_cache(maxsize=1)
def _build_pass1():
    nc = bass.Bass()
    x_ext = nc.declare_dram_parameter("x", [128, 64, C], bf16, isOutput=False)
    a_ext = nc.declare_dram_parameter("a", [128, 64, K], bf16, isOutput=False)
    xp_ext = nc.declare_dram_parameter("xp", [K, C], f32, isOutput=True)

    XCH = 8     # i's per x DMA chunk (128, 8, 256) bf16 = 4KB/partition
    ACH = 32    # i's per a DMA chunk (128, 32, 64) bf16 = 4KB/partition

    with tile.TileContext(nc) as tc:
        with (
            tc.tile_pool(name="xin", bufs=64 // XCH) as xin,
            tc.tile_pool(name="ain", bufs=64 // ACH) as ain,
            tc.tile_pool(name="acc", bufs=1, space="PSUM") as accp,
            tc.tile_pool(name="outp", bufs=1) as outp,
        ):
            qs = [nc.sync, nc.scalar, nc.gpsimd]
            a_sb = []
            for ai in range(64 // ACH):
                ag = ain.tile([128, ACH, K], bf16, tag="a")
                qs[ai % 2].dma_start(
                    out=ag[:], in_=a_ext[:, ai * ACH : (ai + 1) * ACH, :]
                )
                a_sb.append(ag)
            x_sb = []
            for ci in range(64 // XCH):
                xg = xin.tile([128, XCH, C], bf16, tag="x")
                qs[(ci + 2) % 3].dma_start(
                    out=xg[:], in_=x_ext[:, ci * XCH : (ci + 1) * XCH, :]
                )
                x_sb.append(xg)

            acc = accp.tile([K, C], f32)
            for i in range(64):
                nc.tensor.matmul(
                    acc[:],
                    a_sb[i // ACH][:, i % ACH, :],
                    x_sb[i // XCH][:, i % XCH, :],
                    start=(i == 0),
                    stop=(i == 63),
                )
            xps = outp.tile([K, C], f32)
            nc.vector.tensor_copy(xps[:], acc[:])
            nc.sync.dma_start(out=xp_ext[:], in_=xps[:])
    return nc


# --------------------------------------------------------------------------
# Pass 2: full attention for one shard, cluster-major.
# v2: denominator sums computed pre-broadcast (wide indicator lhsT), DVE
# fast-reciprocal for 1/s, one-group software-pipeline skew on the PE.
# --------------------------------------------------------------------------
@lru_cache(maxsize=1)
def _build_pass2(sk_inv: float):
    nc = bass.Bass()
    xs_ext = nc.declare_dram_parameter(
        "xs", [NMG, 128, MG // F, 2, F], fp8, isOutput=False
    )
    as_ext = nc.declare_dram_parameter("as_", [K, NLOC], bf16, isOutput=False)
    wkcl_ext = nc.declare_dram_parameter("wkcl", [128, 4, 2, 128], fp8, isOutput=False)
    cb2_ext = nc.declare_dram_parameter("cb2", [K, 128], bf16, isOutput=False)
    vbd_ext = nc.declare_dram_parameter("vbd", [128, 2, 2, 128], fp8, isOutput=False)
    sind2_ext = nc.declare_dram_parameter("sind2", [128, 2, 128], fp8, isOutput=False)
    wpj_ext = nc.declare_dram_parameter("wpj", [128, 2, C], bf16, isOutput=False)
    y_ext = nc.declare_dram_parameter("y", [2, 128, NLOC], bf16, isOutput=True)

    with tile.TileContext(nc) as tc:
        with (
            tc.tile_pool(name="const", bufs=1) as const,
            tc.tile_pool(name="xt", bufs=NMG) as xtp,
            tc.tile_pool(name="at", bufs=NMG) as atp,
            tc.tile_pool(name="lgp", bufs=2, space="PSUM") as lgp,
            tc.tile_pool(name="spp", bufs=1, space="PSUM") as spp,
            tc.tile_pool(name="xtp2", bufs=2, space="PSUM") as xtp2,
            tc.tile_pool(name="ypp", bufs=2, space="PSUM") as ypp,
            tc.tile_pool(name="pp", bufs=6) as pp,
            tc.tile_pool(name="rp", bufs=4) as rp,
            tc.tile_pool(name="xon", bufs=3) as xon,
            tc.tile_pool(name="ysb", bufs=3) as ysb,
        ):
            wkcl = const.tile([128, 4, 2, 128], fp8)
            nc.sync.dma_start(out=wkcl[:], in_=wkcl_ext[:])
            cb2 = const.tile([K, 128], bf16)
            nc.sync.dma_start(out=cb2[:], in_=cb2_ext[:])

            # first macro-group inputs next, so group 0 can start early;
            # mg0 is split per-gi so the first group's slice lands fastest
            xT_all, aT_all = [], []
            for mg in range(NMG):
                n0 = mg * MG
                xT = xtp.tile([128, MG // F, 2, F], fp8, tag="xT")
                if mg == 0:
                    for gi in range(4):
                        q = nc.sync if gi < 2 else nc.gpsimd
                        q.dma_start(
                            out=xT[:, gi : gi + 1, :, :],
                            in_=xs_ext[mg, :, gi : gi + 1, :, :],
                        )
                else:
                    nc.sync.dma_start(
                        out=xT[:, 0:2, :, :], in_=xs_ext[mg, :, 0:2, :, :]
                    )
                    nc.gpsimd.dma_start(
                        out=xT[:, 2:4, :, :], in_=xs_ext[mg, :, 2:4, :, :]
                    )
                aT = atp.tile([K, MG], bf16, tag="aT")
                nc.sync.dma_start(out=aT[:], in_=as_ext[:, n0 : n0 + MG])
                xT_all.append(xT)
                aT_all.append(aT)
                if mg == 0:
                    vbd = const.tile([128, 2, 2, 128], fp8)
                    nc.gpsimd.dma_start(out=vbd[:], in_=vbd_ext[:])
                    sind2 = const.tile([128, 2, 128], fp8)
                    nc.gpsimd.dma_start(out=sind2[:], in_=sind2_ext[:])
                    wpj = const.tile([128, 2, C], bf16)
                    nc.gpsimd.dma_start(out=wpj[:], in_=wpj_ext[:])

            # phase A: logits chunks (+ cluster bias) -> exp -> Pt (fp8)
            def phase_a(g):
                mg, gi, off = g // 4, g % 4, (g % 4) * F
                xT, aT = xT_all[mg], aT_all[mg]
                Pt = [
                    pp.tile([128, 2, F], fp8, tag="P", name=f"Pt{t}_{g}")
                    for t in range(2)
                ]
                for m in range(4):
                    lg = lgp.tile([128, F], f32, tag="lg")
                    nc.tensor.matmul(
                        lg[:],
                        wkcl[:, m, :, :],
                        xT[:, gi, :, :],
                        start=True, stop=False,
                        perf_mode=DR,
                    )
                    nc.tensor.matmul(
                        lg[:], cb2[:], aT[:, off : off + F],
                        start=False, stop=True,
                    )
                    nc.scalar.activation(
                        Pt[m // 2][:, m % 2, :], lg[:],
                        mybir.ActivationFunctionType.Exp, scale=sk_inv,
                    )
                return Pt

            # phase B1a: denominator + attn@V matmuls (PE only)
            def phase_b1_mm(g, Pt):
                sb2 = spp.tile([128, 2, F], f32, tag="sb")
                for t in range(2):
                    nc.tensor.matmul(
                        sb2[:, t, :], sind2[:], Pt[t][:],
                        start=True, stop=True,
                        perf_mode=DR,
                    )
                Xts = []
                for t in range(2):
                    Xt = xtp2.tile([128, F], f32, tag="Xt")
                    nc.tensor.matmul(
                        Xt[:], vbd[:, t, :, :], Pt[t][:],
                        start=True, stop=True,
                        perf_mode=DR,
                    )
                    Xts.append(Xt)
                return sb2, Xts

            # phase B1b: 1/s (already broadcast) + normalize (scalar + DVE)
            # (ln+exp, NOT Reciprocal: exp/ln/copy share one ACT table set;
            # reciprocal lives in another and thrashes ACT_TABLE_LOAD)
            def phase_b1_fix(g, sb2, Xts):
                lns = rp.tile([128, 2, F], f32, tag="lns")
                nc.scalar.activation(
                    lns[:], sb2[:], mybir.ActivationFunctionType.Ln
                )
                rb = rp.tile([128, 2, F], f32, tag="rb")
                nc.scalar.activation(
                    rb[:], lns[:], mybir.ActivationFunctionType.Exp,
                    scale=-1.0,
                )
                xoutT = xon.tile([128, 2, F], bf16)
                for t in range(2):
                    with nc.allow_low_precision("normalized attn out bf16"):
                        nc.vector.tensor_mul(
                            xoutT[:, t, :], Xts[t][:], rb[:, t, :]
                        )
                return xoutT

            # phase B2: output projection -> y store
            def phase_b2(g, xoutT):
                n0 = g * F
                for cc in range(2):
                    yt = ypp.tile([128, F], f32, tag="y")
                    for ch in range(2):
                        nc.tensor.matmul(
                            yt[:],
                            wpj[:, ch, 128 * cc : 128 * (cc + 1)],
                            xoutT[:, ch, :],
                            start=(ch == 0), stop=(ch == 1),
                        )
                    ys = ysb.tile([128, F], bf16, tag="ys")
                    with nc.allow_low_precision("y store bf16"):
                        nc.vector.tensor_copy(ys[:], yt[:])
                    nc.gpsimd.dma_start(
                        out=y_ext[cc, :, n0 : n0 + F], in_=ys[:]
                    )

            # software pipeline, 2-group skew: PE order per iteration is
            #   [spad'/attnV g-2] [logits g] [proj g-3]
            # so every cross-engine input (exp Pt, normalize mul) has two
            # full iterations of slack and the PE never stalls on them.
            pts = {}
            xos = {}
            pts[0] = phase_a(0)
            pts[1] = phase_a(1)
            for g in range(2, NGROUPS + 2):
                gb = g - 2
                sb2, Xts = phase_b1_mm(gb, pts.pop(gb))
                if g < NGROUPS:
                    pts[g] = phase_a(g)
                xos[gb] = phase_b1_fix(gb, sb2, Xts)
                if gb - 1 in xos:
                    phase_b2(gb - 1, xos.pop(gb - 1))
            phase_b2(NGROUPS - 1, xos.pop(NGROUPS - 1))
    return nc


# --------------------------------------------------------------------------
# Host orchestration
# --------------------------------------------------------------------------
_GLOBAL_SCALES = [2.0 ** 12, 2.0 ** 13]  # (1/Sk placeholder, Sv) - reset below


def _pow2_scale(absmax, target=120.0):
    return float(2.0 ** np.floor(np.log2(target / max(absmax, 1e-30))))


def kernel(
    voxel_features,
    cluster_assignments,
    w_qkv,
    w_proj,
    b_proj,
    cluster_bias,
):
    _install_fixup()
    x_all = np.ascontiguousarray(np.asarray(voxel_features, dtype=np.float32))
    A_all = np.ascontiguousarray(np.asarray(cluster_assignments, dtype=np.float32))
    w_qkv = np.asarray(w_qkv, dtype=np.float32)
    w_proj_np = np.ascontiguousarray(np.asarray(w_proj, dtype=np.float32))
    b_proj_np = np.asarray(b_proj, dtype=np.float32)
    cb = np.asarray(cluster_bias, dtype=np.float32)

    W_q = w_qkv[:, 0:C]
    W_k = w_qkv[:, C : 2 * C]
    W_v = w_qkv[:, 2 * C : 3 * C]

    trace = bool(os.environ.get("BASS_PROFILE"))
    if trace:
        _install_profhook()
    global LAST_EXEC_NS, TRACE_DIRS
    TRACE_DIRS = []

    # ---------------- pass 1 ----------------
    nc1 = _build_pass1()
    in_maps1 = []
    for core in range(NCORES):
        b, half = core // 2, core % 2
        xl = x_all[b, half * NLOC : (half + 1) * NLOC]
        al = A_all[b, half * NLOC : (half + 1) * NLOC]
        in_maps1.append(
            {
                "x": np.ascontiguousarray(xl.reshape(128, 64, C).astype(BF16)),
                "a": np.ascontiguousarray(al.reshape(128, 64, K).astype(BF16)),
            }
        )
    kw1 = {}
    if trace:
        import tempfile
        d = tempfile.mkdtemp(prefix="p1_trace_")
        TRACE_DIRS.append(d)
        kw1 = dict(trace=True, tmpdir=d)
    res1 = run_bass_kernel_spmd(nc1, in_maps1, list(range(NCORES)), **kw1)
    exec1 = getattr(res1, "exec_time_ns", None)
    xp_parts = np.stack([res1.results[c]["xp"] for c in range(NCORES)])  # (8,64,256)

    # ---------------- host glue ----------------
    denom = A_all.sum(axis=1) + EPS  # (B, K)

    Wq3 = W_q.reshape(C, H, D)
    Wkcl_all, v3_all = [], []
    for b in range(B):
        xp = xp_parts[2 * b] + xp_parts[2 * b + 1]
        pooled = xp / denom[b][:, None]
        k_cl = pooled @ W_k
        v_cl = pooled @ W_v
        k3 = k_cl.reshape(K, H, D)
        Wkcl = np.einsum("chd,khd->chk", Wq3, k3).reshape(C, H * K) * SCALE
        Wkcl_all.append(Wkcl)
        v3_all.append(v_cl.reshape(K, H, D))

    Sk = _pow2_scale(max(np.abs(w).max() for w in Wkcl_all))
    Sv = _pow2_scale(max(np.abs(v).max() for v in v3_all))
    _GLOBAL_SCALES[0] = 1.0 / Sk
    _GLOBAL_SCALES[1] = Sv

    # constants shared by all cores
    cb2 = np.zeros((K, 128), np.float32)
    cb2[:, 0:64] = cb * Sk
    cb2[:, 64:128] = cb * Sk
    cb2_bf = cb2.astype(BF16)

    # wide denominator indicator: s lands pre-broadcast in attn-out layout.
    # sind2[p, j, m] = 1 iff 2*j + p//64 == m//32 (same for both P tiles).
    sind2 = np.zeros((128, 2, 128), np.float32)
    for p in range(128):
        for j in range(2):
            blk = 2 * j + p // 64
            sind2[p, j, 32 * blk : 32 * blk + 32] = 1.0
    sind2_8 = sind2.astype(E4)

    wpj = np.ascontiguousarray(
        w_proj_np.reshape(2, 128, C).transpose(1, 0, 2)
    ).astype(BF16)  # [p, ch, c]

    wkcl8_all, vbd8_all = [], []
    for b in range(B):
        # [p, m, ch, mcol]: chunk-m weight blocks with the 2 c-halves adjacent
        wk = (Wkcl_all[b] * Sk).reshape(2, 128, 4, 128).transpose(1, 2, 0, 3)
        wkcl8_all.append(np.ascontiguousarray(wk).astype(E4))
        v3s = v3_all[b] * Sv
        vbd = np.zeros((128, 2, 2, 128), np.float32)
        for t in range(2):
            for jj in range(2):
                for h2 in range(2):
                    c0 = jj * 64 + 32 * h2
                    vbd[64 * h2 : 64 * (h2 + 1), t, jj, c0 : c0 + 32] = v3s[
                        :, 4 * t + 2 * jj + h2, :
                    ]
        vbd8_all.append(vbd.astype(E4))

    # ---------------- pass 2 ----------------
    nc2 = _build_pass2(float(1.0 / Sk))
    in_maps2 = []
    for core in range(NCORES):
        b, half = core // 2, core % 2
        xl = x_all[b, half * NLOC : (half + 1) * NLOC]
        al = A_all[b, half * NLOC : (half + 1) * NLOC]
        in_maps2.append(
            {
                # [mg, p, gi, ch, nf]: per 512-col group, the 2 c-half
                # k-tiles adjacent in the free dim (DoubleRow layout)
                "xs": np.ascontiguousarray(
                    xl.T.reshape(2, 128, NMG, MG // F, F).transpose(2, 1, 3, 0, 4)
                ).astype(E4),
                "as_": np.ascontiguousarray(al.T).astype(BF16),
                "wkcl": wkcl8_all[b],
                "cb2": cb2_bf,
                "vbd": vbd8_all[b],
                "sind2": sind2_8,
                "wpj": wpj,
            }
        )
    kw2 = {}
    if trace:
        import tempfile
        d = tempfile.mkdtemp(prefix="p2_trace_")
        TRACE_DIRS.append(d)
        kw2 = dict(trace=True, tmpdir=d)
    res2 = run_bass_kernel_spmd(nc2, in_maps2, list(range(NCORES)), **kw2)
    exec2 = getattr(res2, "exec_time_ns", None)
    if exec1 is not None and exec2 is not None:
        LAST_EXEC_NS = exec1 + exec2
        globals()["LAST_EXEC_SPLIT"] = (exec1, exec2)

    inv_sv = 1.0 / Sv
    y_out = np.zeros((B, N, C), np.float32)
    for core in range(NCORES):
        b, half = core // 2, core % 2
        yv = res2.results[core]["y"].astype(np.float32)  # (2,128,NLOC), *Sv
        y_out[b, half * NLOC : (half + 1) * NLOC] = (
            yv.transpose(2, 0, 1).reshape(NLOC, C) * inv_sv
        )
    y_out += b_proj_np[None, None, :]
    return y_out



# revision 19
# speedup vs baseline: 1.0329x; 1.0329x over previous
"""ClusterAwareAttention Trainium2 kernel (8 NeuronCores, axon/PJRT path).

Sharding: data-parallel over (batch, sequence-half) -> 8 shards of 8192 rows.

Two launches:
  Pass 1: per-shard cluster pooling partial sums  xp = A_loc^T @ x_loc  (64, 256)
          in bf16 (p-major contiguous layout for full-rate DMA).
  Host:   reduce halves, build pooled K/V-derived constants:
            Wk_cl (x @ Wk_cl = q @ k_cluster^T * scale, folded through W_q),
            VBD block-diagonal v_cluster, cluster-bias, indicators; global
            pow2 scales Sk (fp8 Wk_cl) and Sv (fp8 v_cluster).
  Pass 2: per-shard fused attention, cluster-major (n on the free axis,
          512-col groups):
            fp8 DoubleRow logits (256-deep contraction in one stream)
            + bf16 cluster bias -> exp (fp8 out, scale=1/Sk)
            -> fp8 DoubleRow denominators -> DVE reciprocal (bf16)
            -> bf16 broadcast matmul -> fp8 DoubleRow attn@V
            -> DVE normalize (bf16) -> bf16 output projection
            -> direct PSUM->DRAM y writes (f32; bias + 1/Sv applied on host).

Precision (validated on host): pooling bf16, logits fp8 (scaled), P fp8,
v_cluster fp8 (scaled), everything else bf16/f32 accum. rel err ~8e-3.
"""

import json
import os
from functools import lru_cache

import numpy as np

import concourse.bass as bass
import concourse.tile as tile
from concourse import mybir
from concourse.bass_utils import run_bass_kernel_spmd

import ml_dtypes

BF16 = ml_dtypes.bfloat16
E4 = ml_dtypes.float8_e4m3   # mybir.dt.float8e4 <-> jnp/ml_dtypes float8_e4m3 (max 240)

B, N, C, H, K = 4, 16384, 256, 8, 64
D = C // H
EPS = 1e-8
SCALE = D ** -0.5
NLOC = N // 2           # rows per shard
F = 512                 # group size (n on the free axis)
NGROUPS = NLOC // F     # 16
MG = 2048               # DMA macro-group (4 groups per input DMA chunk)
NMG = NLOC // MG        # 4
NCORES = 8

f32 = mybir.dt.float32
bf16 = mybir.dt.bfloat16
fp8 = mybir.dt.float8e4
DR = mybir.MatmulPerfMode.DoubleRow


# --------------------------------------------------------------------------
# BIR fixup: this container's walrus rejects instructions with >1 sync wait.
# Split extra waits onto single-wait EventSemaphore instructions just before.
# --------------------------------------------------------------------------
def _split_block(bb, counter):
    insts = bb.get("instructions")
    if insts:
        new_insts = []
        for inst in insts:
            si = inst.get("sync_info") or {}
            waits = si.get("on_wait") or []
            if len(waits) > 1:
                for w in waits[:-1]:
                    counter[0] += 1
                    new_insts.append(
                        {
                            "debug": inst.get("debug", 0),
                            "engine": inst["engine"],
                            "ins": [],
                            "name": f"WSPLIT-{counter[0]}",
                            "opcode": "EventSemaphore",
                            "outs": [],
                            "sync_info": {"on_update": [], "on_wait": [w]},
                        }
                    )
                si = dict(si)
                si["on_wait"] = [waits[-1]]
                inst = dict(inst)
                inst["sync_info"] = si
            new_insts.append(inst)
        bb["instructions"] = new_insts
    for sub in bb.get("blocks", []) or []:
        _split_block(sub, counter)


def _fixup_bir_json(bir_json: bytes) -> bytes:
    bir = json.loads(bir_json)
    counter = [0]
    for fn in bir.get("functions", []):
        for bb in fn.get("blocks", []) or []:
            _split_block(bb, counter)
    return json.dumps(bir).encode()


LAST_EXEC_NS = None
TRACE_DIRS = []


def _scalar_recip(nc, out, in_):
    """Raw ACT Reciprocal (bass blocks the wrapper for accuracy reasons;
    our 1/s operands are well-conditioned and the rel-err gate validates)."""
    se = nc.scalar
    ins = [se.lower_ap(in_)]
    for v in (0.0, 1.0, 0.0):  # bias, scale, alpha
        ins.append(mybir.ImmediateValue(dtype=mybir.dt.float32, value=v))
    return se.add_instruction(
        mybir.InstActivation(
            name=nc.get_next_instruction_name(),
            func=mybir.ActivationFunctionType.Reciprocal,
            ins=ins,
            outs=[se.lower_ap(out)],
        )
    )


def _install_profhook():
    import sys
    import types

    if "antenv.axon_hooks" in sys.modules:
        return
    import antenv

    mod = types.ModuleType("antenv.axon_hooks")
    _hook = [None]
    mod.set_axon_ntff_profile_hook = lambda h: _hook.__setitem__(0, h)
    mod.get_axon_ntff_profile_hook = lambda: _hook[0]
    sys.modules["antenv.axon_hooks"] = mod
    antenv.axon_hooks = mod
    from trn_agent_boot.trn_boot import _ntff_profile_via_ctypes

    mod.set_axon_ntff_profile_hook(
        _ntff_profile_via_ctypes("/opt/axon/libaxon_pjrt.so")
    )


_fixup_installed = False


def _install_fixup():
    global _fixup_installed
    if _fixup_installed:
        return
    _fixup_installed = True
    import concourse.bass_utils as bu
    import concourse.bass2jax as b2j

    orig = bu.compile_bir_kernel

    def patched(bir_json, tmpdir, neff_name="file.neff"):
        return orig(_fixup_bir_json(bir_json), tmpdir, neff_name=neff_name)

    bu.compile_bir_kernel = patched
    b2j.compile_bir_kernel = patched

    if os.environ.get("BASS_LDW_OPT"):
        orig_run = bu.run_command

        def patched_run(cmd, *a, **kw):
            cmd = [
                c.replace("--enable-ldw-opt=false", "--enable-ldw-opt=true")
                if isinstance(c, str)
                else c
                for c in cmd
            ]
            return orig_run(cmd, *a, **kw)

        bu.run_command = patched_run


# --------------------------------------------------------------------------
# Pass 1: xp[k, c] = sum_n A_loc[n, k] * x_loc[n, c], bf16 in / f32 accum.
# p-major layout: row n = 64*p + i  ->  x[p, i, c] contiguous per partition.
# --------------------------------------------------------------------------
@lru_cache(maxsize=1)
def _build_pass1():
    nc = bass.Bass()
    x_ext = nc.declare_dram_parameter("x", [128, 64, C], bf16, isOutput=False)
    a_ext = nc.declare_dram_parameter("a", [128, 64, K], bf16, isOutput=False)
    xp_ext = nc.declare_dram_parameter("xp", [K, C], f32, isOutput=True)

    XCH = 8     # i's per x DMA chunk (128, 8, 256) bf16 = 4KB/partition
    ACH = 8     # i's per a DMA chunk (128, 8, 64) bf16 = 1KB/partition

    with tile.TileContext(nc) as tc:
        with (
            tc.tile_pool(name="xin", bufs=64 // XCH) as xin,
            tc.tile_pool(name="ain", bufs=64 // ACH) as ain,
            tc.tile_pool(name="acc", bufs=1, space="PSUM") as accp,
            tc.tile_pool(name="outp", bufs=1) as outp,
        ):
            qs = [nc.sync, nc.scalar, nc.gpsimd]
            # interleave a/x chunk loads so matmul i can start as soon as
            # chunk i//8 of BOTH streams has landed
            a_sb, x_sb = [], []
            for ci in range(64 // XCH):
                ag = ain.tile([128, ACH, K], bf16, tag="a")
                qs[(2 * ci) % 3].dma_start(
                    out=ag[:], in_=a_ext[:, ci * ACH : (ci + 1) * ACH, :]
                )
                a_sb.append(ag)
                xg = xin.tile([128, XCH, C], bf16, tag="x")
                qs[(2 * ci + 1) % 3].dma_start(
                    out=xg[:], in_=x_ext[:, ci * XCH : (ci + 1) * XCH, :]
                )
                x_sb.append(xg)

            acc = accp.tile([K, C], f32)
            for i in range(64):
                nc.tensor.matmul(
                    acc[:],
                    a_sb[i // ACH][:, i % ACH, :],
                    x_sb[i // XCH][:, i % XCH, :],
                    start=(i == 0),
                    stop=(i == 63),
                )
            xps = outp.tile([K, C], f32)
            nc.vector.tensor_copy(xps[:], acc[:])
            nc.sync.dma_start(out=xp_ext[:], in_=xps[:])
    return nc


# --------------------------------------------------------------------------
# Pass 2: full attention for one shard, cluster-major.
# v2: denominator sums computed pre-broadcast (wide indicator lhsT), DVE
# fast-reciprocal for 1/s, one-group software-pipeline skew on the PE.
# --------------------------------------------------------------------------
@lru_cache(maxsize=1)
def _build_pass2(sk_inv: float):
    nc = bass.Bass()
    xs_ext = nc.declare_dram_parameter(
        "xs", [NMG, 128, MG // F, 2, F], fp8, isOutput=False
    )
    as_ext = nc.declare_dram_parameter("as_", [K, NLOC], bf16, isOutput=False)
    wkcl_ext = nc.declare_dram_parameter("wkcl", [128, 4, 2, 128], fp8, isOutput=False)
    cb2_ext = nc.declare_dram_parameter("cb2", [K, 128], bf16, isOutput=False)
    vbd_ext = nc.declare_dram_parameter("vbd", [128, 2, 2, 128], fp8, isOutput=False)
    sind2_ext = nc.declare_dram_parameter("sind2", [128, 2, 128], fp8, isOutput=False)
    wpj_ext = nc.declare_dram_parameter("wpj", [128, 2, C], bf16, isOutput=False)
    y_ext = nc.declare_dram_parameter("y", [2, 128, NLOC], bf16, isOutput=True)

    with tile.TileContext(nc) as tc:
        with (
            tc.tile_pool(name="const", bufs=1) as const,
            tc.tile_pool(name="xt", bufs=NMG) as xtp,
            tc.tile_pool(name="at", bufs=NMG) as atp,
            tc.tile_pool(name="lgp", bufs=2, space="PSUM") as lgp,
            tc.tile_pool(name="spp", bufs=1, space="PSUM") as spp,
            tc.tile_pool(name="xtp2", bufs=2, space="PSUM") as xtp2,
            tc.tile_pool(name="ypp", bufs=2, space="PSUM") as ypp,
            tc.tile_pool(name="pp", bufs=6) as pp,
            tc.tile_pool(name="rp", bufs=4) as rp,
            tc.tile_pool(name="xon", bufs=3) as xon,
            tc.tile_pool(name="ysb", bufs=3) as ysb,
        ):
            wkcl = const.tile([128, 4, 2, 128], fp8)
            nc.sync.dma_start(out=wkcl[:], in_=wkcl_ext[:])
            cb2 = const.tile([K, 128], bf16)
            nc.sync.dma_start(out=cb2[:], in_=cb2_ext[:])

            # first macro-group inputs next, so group 0 can start early;
            # mg0 is split per-gi so the first group's slice lands fastest
            xT_all, aT_all = [], []
            for mg in range(NMG):
                n0 = mg * MG
                xT = xtp.tile([128, MG // F, 2, F], fp8, tag="xT")
                if mg == 0:
                    for gi in range(4):
                        q = nc.sync if gi < 2 else nc.gpsimd
                        q.dma_start(
                            out=xT[:, gi : gi + 1, :, :],
                            in_=xs_ext[mg, :, gi : gi + 1, :, :],
                        )
                else:
                    nc.sync.dma_start(
                        out=xT[:, 0:2, :, :], in_=xs_ext[mg, :, 0:2, :, :]
                    )
                    nc.gpsimd.dma_start(
                        out=xT[:, 2:4, :, :], in_=xs_ext[mg, :, 2:4, :, :]
                    )
                aT = atp.tile([K, MG], bf16, tag="aT")
                nc.sync.dma_start(out=aT[:], in_=as_ext[:, n0 : n0 + MG])
                xT_all.append(xT)
                aT_all.append(aT)
                if mg == 0:
                    vbd = const.tile([128, 2, 2, 128], fp8)
                    nc.gpsimd.dma_start(out=vbd[:], in_=vbd_ext[:])
                    sind2 = const.tile([128, 2, 128], fp8)
                    nc.gpsimd.dma_start(out=sind2[:], in_=sind2_ext[:])
                    wpj = const.tile([128, 2, C], bf16)
                    nc.gpsimd.dma_start(out=wpj[:], in_=wpj_ext[:])

            # phase A: logits chunks (+ cluster bias) -> exp -> Pt (fp8)
            def phase_a(g):
                mg, gi, off = g // 4, g % 4, (g % 4) * F
                xT, aT = xT_all[mg], aT_all[mg]
                Pt = [
                    pp.tile([128, 2, F], fp8, tag="P", name=f"Pt{t}_{g}")
                    for t in range(2)
                ]
                for m in range(4):
                    lg = lgp.tile([128, F], f32, tag="lg")
                    nc.tensor.matmul(
                        lg[:],
                        wkcl[:, m, :, :],
                        xT[:, gi, :, :],
                        start=True, stop=False,
                        perf_mode=DR,
                    )
                    nc.tensor.matmul(
                        lg[:], cb2[:], aT[:, off : off + F],
                        start=False, stop=True,
                    )
                    nc.scalar.activation(
                        Pt[m // 2][:, m % 2, :], lg[:],
                        mybir.ActivationFunctionType.Exp, scale=sk_inv,
                    )
                return Pt

            # phase B1a: denominator + attn@V matmuls (PE only)
            def phase_b1_mm(g, Pt):
                sb2 = spp.tile([128, 2, F], f32, tag="sb")
                for t in range(2):
                    nc.tensor.matmul(
                        sb2[:, t, :], sind2[:], Pt[t][:],
                        start=True, stop=True,
                        perf_mode=DR,
                    )
                Xts = []
                for t in range(2):
                    Xt = xtp2.tile([128, F], f32, tag="Xt")
                    nc.tensor.matmul(
                        Xt[:], vbd[:, t, :, :], Pt[t][:],
                        start=True, stop=True,
                        perf_mode=DR,
                    )
                    Xts.append(Xt)
                return sb2, Xts

            # phase B1b: 1/s (already broadcast) + normalize (scalar + DVE)
            # (ln+exp, NOT Reciprocal: exp/ln/copy share one ACT table set;
            # reciprocal lives in another and thrashes ACT_TABLE_LOAD)
            def phase_b1_fix(g, sb2, Xts):
                lns = rp.tile([128, 2, F], f32, tag="lns")
                nc.scalar.activation(
                    lns[:], sb2[:], mybir.ActivationFunctionType.Ln
                )
                rb = rp.tile([128, 2, F], f32, tag="rb")
                nc.scalar.activation(
                    rb[:], lns[:], mybir.ActivationFunctionType.Exp,
                    scale=-1.0,
                )
                xoutT = xon.tile([128, 2, F], bf16)
                for t in range(2):
                    with nc.allow_low_precision("normalized attn out bf16"):
                        nc.vector.tensor_mul(
                            xoutT[:, t, :], Xts[t][:], rb[:, t, :]
                        )
                return xoutT

            # phase B2: output projection -> y store
            def phase_b2(g, xoutT):
                n0 = g * F
                for cc in range(2):
                    yt = ypp.tile([128, F], f32, tag="y")
                    for ch in range(2):
                        nc.tensor.matmul(
                            yt[:],
                            wpj[:, ch, 128 * cc : 128 * (cc + 1)],
                            xoutT[:, ch, :],
                            start=(ch == 0), stop=(ch == 1),
                        )
                    ys = ysb.tile([128, F], bf16, tag="ys")
                    with nc.allow_low_precision("y store bf16"):
                        nc.vector.tensor_copy(ys[:], yt[:])
                    nc.gpsimd.dma_start(
                        out=y_ext[cc, :, n0 : n0 + F], in_=ys[:]
                    )

            # software pipeline: PE order per iteration is
            #   [spad'/attnV g-1] [logits g] [proj g-2]
            # and the scalar queue sees exps(g) before recip(g-1), so the
            # next group's exps are never stuck behind the normalizer.
            prev_pt = phase_a(0)
            prev_xo = None
            for g in range(1, NGROUPS):
                sb2, Xts = phase_b1_mm(g - 1, prev_pt)
                cur_pt = phase_a(g)
                cur_xo = phase_b1_fix(g - 1, sb2, Xts)
                if prev_xo is not None:
                    phase_b2(g - 2, prev_xo)
                prev_pt, prev_xo = cur_pt, cur_xo
            sb2, Xts = phase_b1_mm(NGROUPS - 1, prev_pt)
            cur_xo = phase_b1_fix(NGROUPS - 1, sb2, Xts)
            phase_b2(NGROUPS - 2, prev_xo)
            phase_b2(NGROUPS - 1, cur_xo)
    return nc


# --------------------------------------------------------------------------
# Host orchestration
# --------------------------------------------------------------------------
_GLOBAL_SCALES = [2.0 ** 12, 2.0 ** 13]  # (1/Sk placeholder, Sv) - reset below


def _pow2_scale(absmax, target=120.0):
    return float(2.0 ** np.floor(np.log2(target / max(absmax, 1e-30))))


def kernel(
    voxel_features,
    cluster_assignments,
    w_qkv,
    w_proj,
    b_proj,
    cluster_bias,
):
    _install_fixup()
    x_all = np.ascontiguousarray(np.asarray(voxel_features, dtype=np.float32))
    A_all = np.ascontiguousarray(np.asarray(cluster_assignments, dtype=np.float32))
    w_qkv = np.asarray(w_qkv, dtype=np.float32)
    w_proj_np = np.ascontiguousarray(np.asarray(w_proj, dtype=np.float32))
    b_proj_np = np.asarray(b_proj, dtype=np.float32)
    cb = np.asarray(cluster_bias, dtype=np.float32)

    W_q = w_qkv[:, 0:C]
    W_k = w_qkv[:, C : 2 * C]
    W_v = w_qkv[:, 2 * C : 3 * C]

    trace = bool(os.environ.get("BASS_PROFILE"))
    if trace:
        _install_profhook()
    global LAST_EXEC_NS, TRACE_DIRS
    TRACE_DIRS = []

    # ---------------- pass 1 ----------------
    nc1 = _build_pass1()
    in_maps1 = []
    for core in range(NCORES):
        b, half = core // 2, core % 2
        xl = x_all[b, half * NLOC : (half + 1) * NLOC]
        al = A_all[b, half * NLOC : (half + 1) * NLOC]
        in_maps1.append(
            {
                "x": np.ascontiguousarray(xl.reshape(128, 64, C).astype(BF16)),
                "a": np.ascontiguousarray(al.reshape(128, 64, K).astype(BF16)),
            }
        )
    kw1 = {}
    if trace:
        import tempfile
        d = tempfile.mkdtemp(prefix="p1_trace_")
        TRACE_DIRS.append(d)
        kw1 = dict(trace=True, tmpdir=d)
    res1 = run_bass_kernel_spmd(nc1, in_maps1, list(range(NCORES)), **kw1)
    exec1 = getattr(res1, "exec_time_ns", None)
    xp_parts = np.stack([res1.results[c]["xp"] for c in range(NCORES)])  # (8,64,256)

    # ---------------- host glue ----------------
    denom = A_all.sum(axis=1) + EPS  # (B, K)

    Wq3 = W_q.reshape(C, H, D)
    Wkcl_all, v3_all = [], []
    for b in range(B):
        xp = xp_parts[2 * b] + xp_parts[2 * b + 1]
        pooled = xp / denom[b][:, None]
        k_cl = pooled @ W_k
        v_cl = pooled @ W_v
        k3 = k_cl.reshape(K, H, D)
        Wkcl = np.einsum("chd,khd->chk", Wq3, k3).reshape(C, H * K) * SCALE
        Wkcl_all.append(Wkcl)
        v3_all.append(v_cl.reshape(K, H, D))

    Sk = _pow2_scale(max(np.abs(w).max() for w in Wkcl_all))
    Sv = _pow2_scale(max(np.abs(v).max() for v in v3_all))
    _GLOBAL_SCALES[0] = 1.0 / Sk
    _GLOBAL_SCALES[1] = Sv

    # constants shared by all cores
    cb2 = np.zeros((K, 128), np.float32)
    cb2[:, 0:64] = cb * Sk
    cb2[:, 64:128] = cb * Sk
    cb2_bf = cb2.astype(BF16)

    # wide denominator indicator: s lands pre-broadcast in attn-out layout.
    # sind2[p, j, m] = 1 iff 2*j + p//64 == m//32 (same for both P tiles).
    sind2 = np.zeros((128, 2, 128), np.float32)
    for p in range(128):
        for j in range(2):
            blk = 2 * j + p // 64
            sind2[p, j, 32 * blk : 32 * blk + 32] = 1.0
    sind2_8 = sind2.astype(E4)

    wpj = np.ascontiguousarray(
        w_proj_np.reshape(2, 128, C).transpose(1, 0, 2)
    ).astype(BF16)  # [p, ch, c]

    wkcl8_all, vbd8_all = [], []
    for b in range(B):
        # [p, m, ch, mcol]: chunk-m weight blocks with the 2 c-halves adjacent
        wk = (Wkcl_all[b] * Sk).reshape(2, 128, 4, 128).transpose(1, 2, 0, 3)
        wkcl8_all.append(np.ascontiguousarray(wk).astype(E4))
        v3s = v3_all[b] * Sv
        vbd = np.zeros((128, 2, 2, 128), np.float32)
        for t in range(2):
            for jj in range(2):
                for h2 in range(2):
                    c0 = jj * 64 + 32 * h2
                    vbd[64 * h2 : 64 * (h2 + 1), t, jj, c0 : c0 + 32] = v3s[
                        :, 4 * t + 2 * jj + h2, :
                    ]
        vbd8_all.append(vbd.astype(E4))

    # ---------------- pass 2 ----------------
    nc2 = _build_pass2(float(1.0 / Sk))
    in_maps2 = []
    for core in range(NCORES):
        b, half = core // 2, core % 2
        xl = x_all[b, half * NLOC : (half + 1) * NLOC]
        al = A_all[b, half * NLOC : (half + 1) * NLOC]
        in_maps2.append(
            {
                # [mg, p, gi, ch, nf]: per 512-col group, the 2 c-half
                # k-tiles adjacent in the free dim (DoubleRow layout)
                "xs": np.ascontiguousarray(
                    xl.T.reshape(2, 128, NMG, MG // F, F).transpose(2, 1, 3, 0, 4)
                ).astype(E4),
                "as_": np.ascontiguousarray(al.T).astype(BF16),
                "wkcl": wkcl8_all[b],
                "cb2": cb2_bf,
                "vbd": vbd8_all[b],
                "sind2": sind2_8,
                "wpj": wpj,
            }
        )
    kw2 = {}
    if trace:
        import tempfile
        d = tempfile.mkdtemp(prefix="p2_trace_")
        TRACE_DIRS.append(d)
        kw2 = dict(trace=True, tmpdir=d)
    res2 = run_bass_kernel_spmd(nc2, in_maps2, list(range(NCORES)), **kw2)
    exec2 = getattr(res2, "exec_time_ns", None)
    if exec1 is not None and exec2 is not None:
        LAST_EXEC_NS = exec1 + exec2
        globals()["LAST_EXEC_SPLIT"] = (exec1, exec2)

    inv_sv = 1.0 / Sv
    y_out = np.zeros((B, N, C), np.float32)
    for core in range(NCORES):
        b, half = core // 2, core % 2
        yv = res2.results[core]["y"].astype(np.float32)  # (2,128,NLOC), *Sv
        y_out[b, half * NLOC : (half + 1) * NLOC] = (
            yv.transpose(2, 0, 1).reshape(NLOC, C) * inv_sv
        )
    y_out += b_proj_np[None, None, :]
    return y_out



# revision 22
# speedup vs baseline: 1.0362x; 1.0032x over previous
"""ClusterAwareAttention Trainium2 kernel (8 NeuronCores, axon/PJRT path).

Sharding: data-parallel over (batch, sequence-half) -> 8 shards of 8192 rows.

Two launches:
  Pass 1: per-shard cluster pooling partial sums  xp = A_loc^T @ x_loc  (64, 256)
          in bf16 (p-major contiguous layout for full-rate DMA).
  Host:   reduce halves, build pooled K/V-derived constants:
            Wk_cl (x @ Wk_cl = q @ k_cluster^T * scale, folded through W_q),
            VBD block-diagonal v_cluster, cluster-bias, indicators; global
            pow2 scales Sk (fp8 Wk_cl) and Sv (fp8 v_cluster).
  Pass 2: per-shard fused attention, cluster-major (n on the free axis,
          512-col groups):
            fp8 DoubleRow logits (256-deep contraction in one stream)
            + bf16 cluster bias -> exp (fp8 out, scale=1/Sk)
            -> fp8 DoubleRow denominators -> DVE reciprocal (bf16)
            -> bf16 broadcast matmul -> fp8 DoubleRow attn@V
            -> DVE normalize (bf16) -> bf16 output projection
            -> direct PSUM->DRAM y writes (f32; bias + 1/Sv applied on host).

Precision (validated on host): pooling bf16, logits fp8 (scaled), P fp8,
v_cluster fp8 (scaled), everything else bf16/f32 accum. rel err ~8e-3.
"""

import json
import os
from functools import lru_cache

import numpy as np

import concourse.bass as bass
import concourse.tile as tile
from concourse import mybir
from concourse.bass_utils import run_bass_kernel_spmd

import ml_dtypes

BF16 = ml_dtypes.bfloat16
E4 = ml_dtypes.float8_e4m3   # mybir.dt.float8e4 <-> jnp/ml_dtypes float8_e4m3 (max 240)

B, N, C, H, K = 4, 16384, 256, 8, 64
D = C // H
EPS = 1e-8
SCALE = D ** -0.5
NLOC = N // 2           # rows per shard
F = 512                 # group size (n on the free axis)
NGROUPS = NLOC // F     # 16
MG = 2048               # DMA macro-group (4 groups per input DMA chunk)
NMG = NLOC // MG        # 4
NCORES = 8

f32 = mybir.dt.float32
bf16 = mybir.dt.bfloat16
fp8 = mybir.dt.float8e4
DR = mybir.MatmulPerfMode.DoubleRow


# --------------------------------------------------------------------------
# BIR fixup: this container's walrus rejects instructions with >1 sync wait.
# Split extra waits onto single-wait EventSemaphore instructions just before.
# --------------------------------------------------------------------------
def _split_block(bb, counter):
    insts = bb.get("instructions")
    if insts:
        new_insts = []
        for inst in insts:
            si = inst.get("sync_info") or {}
            waits = si.get("on_wait") or []
            if len(waits) > 1:
                for w in waits[:-1]:
                    counter[0] += 1
                    new_insts.append(
                        {
                            "debug": inst.get("debug", 0),
                            "engine": inst["engine"],
                            "ins": [],
                            "name": f"WSPLIT-{counter[0]}",
                            "opcode": "EventSemaphore",
                            "outs": [],
                            "sync_info": {"on_update": [], "on_wait": [w]},
                        }
                    )
                si = dict(si)
                si["on_wait"] = [waits[-1]]
                inst = dict(inst)
                inst["sync_info"] = si
            new_insts.append(inst)
        bb["instructions"] = new_insts
    for sub in bb.get("blocks", []) or []:
        _split_block(sub, counter)


def _fixup_bir_json(bir_json: bytes) -> bytes:
    bir = json.loads(bir_json)
    counter = [0]
    for fn in bir.get("functions", []):
        for bb in fn.get("blocks", []) or []:
            _split_block(bb, counter)
    return json.dumps(bir).encode()


LAST_EXEC_NS = None
TRACE_DIRS = []


def _scalar_recip(nc, out, in_):
    """Raw ACT Reciprocal (bass blocks the wrapper for accuracy reasons;
    our 1/s operands are well-conditioned and the rel-err gate validates)."""
    se = nc.scalar
    ins = [se.lower_ap(in_)]
    for v in (0.0, 1.0, 0.0):  # bias, scale, alpha
        ins.append(mybir.ImmediateValue(dtype=mybir.dt.float32, value=v))
    return se.add_instruction(
        mybir.InstActivation(
            name=nc.get_next_instruction_name(),
            func=mybir.ActivationFunctionType.Reciprocal,
            ins=ins,
            outs=[se.lower_ap(out)],
        )
    )


def _install_profhook():
    import sys
    import types

    if "antenv.axon_hooks" in sys.modules:
        return
    import antenv

    mod = types.ModuleType("antenv.axon_hooks")
    _hook = [None]
    mod.set_axon_ntff_profile_hook = lambda h: _hook.__setitem__(0, h)
    mod.get_axon_ntff_profile_hook = lambda: _hook[0]
    sys.modules["antenv.axon_hooks"] = mod
    antenv.axon_hooks = mod
    from trn_agent_boot.trn_boot import _ntff_profile_via_ctypes

    mod.set_axon_ntff_profile_hook(
        _ntff_profile_via_ctypes("/opt/axon/libaxon_pjrt.so")
    )


_fixup_installed = False


def _install_fixup():
    global _fixup_installed
    if _fixup_installed:
        return
    _fixup_installed = True
    import concourse.bass_utils as bu
    import concourse.bass2jax as b2j

    orig = bu.compile_bir_kernel

    def patched(bir_json, tmpdir, neff_name="file.neff"):
        return orig(_fixup_bir_json(bir_json), tmpdir, neff_name=neff_name)

    bu.compile_bir_kernel = patched
    b2j.compile_bir_kernel = patched

    if os.environ.get("BASS_LDW_OPT"):
        orig_run = bu.run_command

        def patched_run(cmd, *a, **kw):
            cmd = [
                c.replace("--enable-ldw-opt=false", "--enable-ldw-opt=true")
                if isinstance(c, str)
                else c
                for c in cmd
            ]
            return orig_run(cmd, *a, **kw)

        bu.run_command = patched_run


# --------------------------------------------------------------------------
# Pass 1: xp[k, c] = sum_n A_loc[n, k] * x_loc[n, c], bf16 in / f32 accum.
# p-major layout: row n = 64*p + i  ->  x[p, i, c] contiguous per partition.
# --------------------------------------------------------------------------
@lru_cache(maxsize=1)
def _build_pass1():
    nc = bass.Bass()
    x_ext = nc.declare_dram_parameter("x", [128, 64, C], bf16, isOutput=False)
    a_ext = nc.declare_dram_parameter("a", [128, 64, K], bf16, isOutput=False)
    xp_ext = nc.declare_dram_parameter("xp", [K, C], f32, isOutput=True)

    XCH = 8     # i's per x DMA chunk (128, 8, 256) bf16 = 4KB/partition
    ACH = 8     # i's per a DMA chunk (128, 8, 64) bf16 = 1KB/partition

    with tile.TileContext(nc) as tc:
        with (
            tc.tile_pool(name="xin", bufs=64 // XCH) as xin,
            tc.tile_pool(name="ain", bufs=64 // ACH) as ain,
            tc.tile_pool(name="acc", bufs=1, space="PSUM") as accp,
            tc.tile_pool(name="outp", bufs=1) as outp,
        ):
            qs = [nc.sync, nc.scalar, nc.gpsimd]
            # interleave a/x chunk loads so matmul i can start as soon as
            # chunk i//8 of BOTH streams has landed
            a_sb, x_sb = [], []
            for ci in range(64 // XCH):
                ag = ain.tile([128, ACH, K], bf16, tag="a")
                qs[(2 * ci) % 3].dma_start(
                    out=ag[:], in_=a_ext[:, ci * ACH : (ci + 1) * ACH, :]
                )
                a_sb.append(ag)
                xg = xin.tile([128, XCH, C], bf16, tag="x")
                qs[(2 * ci + 1) % 3].dma_start(
                    out=xg[:], in_=x_ext[:, ci * XCH : (ci + 1) * XCH, :]
                )
                x_sb.append(xg)

            acc = accp.tile([K, C], f32)
            for i in range(64):
                nc.tensor.matmul(
                    acc[:],
                    a_sb[i // ACH][:, i % ACH, :],
                    x_sb[i // XCH][:, i % XCH, :],
                    start=(i == 0),
                    stop=(i == 63),
                )
            xps = outp.tile([K, C], f32)
            nc.vector.tensor_copy(xps[:], acc[:])
            nc.sync.dma_start(out=xp_ext[:], in_=xps[:])
    return nc


# --------------------------------------------------------------------------
# Pass 2: full attention for one shard, cluster-major.
# v2: denominator sums computed pre-broadcast (wide indicator lhsT), DVE
# fast-reciprocal for 1/s, one-group software-pipeline skew on the PE.
# --------------------------------------------------------------------------
@lru_cache(maxsize=1)
def _build_pass2(sk_inv: float):
    nc = bass.Bass()
    xs_ext = nc.declare_dram_parameter(
        "xs", [NMG, 128, MG // F, 2, F], fp8, isOutput=False
    )
    as_ext = nc.declare_dram_parameter("as_", [K, NLOC], bf16, isOutput=False)
    wkcl_ext = nc.declare_dram_parameter("wkcl", [128, 4, 2, 128], fp8, isOutput=False)
    cb2_ext = nc.declare_dram_parameter("cb2", [K, 128], bf16, isOutput=False)
    vbd_ext = nc.declare_dram_parameter("vbd", [128, 2, 2, 128], fp8, isOutput=False)
    sind2_ext = nc.declare_dram_parameter("sind2", [128, 2, 128], fp8, isOutput=False)
    wpj_ext = nc.declare_dram_parameter("wpj", [128, 2, C], bf16, isOutput=False)
    y_ext = nc.declare_dram_parameter("y", [2, 128, NLOC], bf16, isOutput=True)

    with tile.TileContext(nc) as tc:
        with (
            tc.tile_pool(name="const", bufs=1) as const,
            tc.tile_pool(name="xt", bufs=NMG) as xtp,
            tc.tile_pool(name="at", bufs=NMG) as atp,
            tc.tile_pool(name="lgp", bufs=2, space="PSUM") as lgp,
            tc.tile_pool(name="spp", bufs=1, space="PSUM") as spp,
            tc.tile_pool(name="xtp2", bufs=2, space="PSUM") as xtp2,
            tc.tile_pool(name="ypp", bufs=2, space="PSUM") as ypp,
            tc.tile_pool(name="pp", bufs=3) as pp,
            tc.tile_pool(name="rp", bufs=4) as rp,
            tc.tile_pool(name="xon", bufs=3) as xon,
            tc.tile_pool(name="ysb", bufs=3) as ysb,
        ):
            wkcl = const.tile([128, 4, 2, 128], fp8)
            nc.sync.dma_start(out=wkcl[:], in_=wkcl_ext[:])
            cb2 = const.tile([K, 128], bf16)
            nc.sync.dma_start(out=cb2[:], in_=cb2_ext[:])

            # inputs split per-gi: each group's slice is one DMA, so every
            # logits matmul waits on exactly one transfer
            qrot = [nc.sync, nc.scalar, nc.gpsimd]
            xT_all, aT_all = [], []
            for mg in range(NMG):
                n0 = mg * MG
                xT = xtp.tile([128, MG // F, 2, F], fp8, tag="xT")
                for gi in range(4):
                    qrot[(4 * mg + gi) % 3].dma_start(
                        out=xT[:, gi : gi + 1, :, :],
                        in_=xs_ext[mg, :, gi : gi + 1, :, :],
                    )
                aT = atp.tile([K, MG], bf16, tag="aT")
                qrot[(4 * mg) % 3].dma_start(
                    out=aT[:], in_=as_ext[:, n0 : n0 + MG]
                )
                xT_all.append(xT)
                aT_all.append(aT)
                if mg == 0:
                    vbd = const.tile([128, 2, 2, 128], fp8)
                    nc.gpsimd.dma_start(out=vbd[:], in_=vbd_ext[:])
                    sind2 = const.tile([128, 2, 128], fp8)
                    nc.gpsimd.dma_start(out=sind2[:], in_=sind2_ext[:])
                    wpj = const.tile([128, 2, C], bf16)
                    nc.gpsimd.dma_start(out=wpj[:], in_=wpj_ext[:])

            # phase A: logits chunks (+ cluster bias) -> exp -> Pt (fp8)
            def phase_a(g):
                mg, gi, off = g // 4, g % 4, (g % 4) * F
                xT, aT = xT_all[mg], aT_all[mg]
                Pt4 = pp.tile([128, 4, F], fp8, tag="P", name=f"Pt_{g}")
                for m in range(4):
                    lg = lgp.tile([128, F], f32, tag="lg")
                    nc.tensor.matmul(
                        lg[:],
                        wkcl[:, m, :, :],
                        xT[:, gi, :, :],
                        start=True, stop=False,
                        perf_mode=DR,
                    )
                    nc.tensor.matmul(
                        lg[:], cb2[:], aT[:, off : off + F],
                        start=False, stop=True,
                    )
                    nc.scalar.activation(
                        Pt4[:, m, :], lg[:],
                        mybir.ActivationFunctionType.Exp, scale=sk_inv,
                    )
                return [Pt4[:, 0:2, :], Pt4[:, 2:4, :]]

            # phase B1a: denominator + attn@V matmuls (PE only)
            def phase_b1_mm(g, Pt):
                sb2 = spp.tile([128, 2, F], f32, tag="sb")
                for t in range(2):
                    nc.tensor.matmul(
                        sb2[:, t, :], sind2[:], Pt[t][:],
                        start=True, stop=True,
                        perf_mode=DR,
                    )
                Xts = []
                for t in range(2):
                    Xt = xtp2.tile([128, F], f32, tag="Xt")
                    nc.tensor.matmul(
                        Xt[:], vbd[:, t, :, :], Pt[t][:],
                        start=True, stop=True,
                        perf_mode=DR,
                    )
                    Xts.append(Xt)
                return sb2, Xts

            # phase B1b: 1/s (already broadcast) + normalize (scalar + DVE)
            # (ln+exp, NOT Reciprocal: exp/ln/copy share one ACT table set;
            # reciprocal lives in another and thrashes ACT_TABLE_LOAD)
            def phase_b1_fix(g, sb2, Xts):
                lns = rp.tile([128, 2, F], f32, tag="lns")
                nc.scalar.activation(
                    lns[:], sb2[:], mybir.ActivationFunctionType.Ln
                )
                rb = rp.tile([128, 2, F], f32, tag="rb")
                nc.scalar.activation(
                    rb[:], lns[:], mybir.ActivationFunctionType.Exp,
                    scale=-1.0,
                )
                xoutT = xon.tile([128, 2, F], bf16)
                for t in range(2):
                    with nc.allow_low_precision("normalized attn out bf16"):
                        nc.vector.tensor_mul(
                            xoutT[:, t, :], Xts[t][:], rb[:, t, :]
                        )
                return xoutT

            # phase B2: output projection -> y store
            def phase_b2(g, xoutT):
                n0 = g * F
                for cc in range(2):
                    yt = ypp.tile([128, F], f32, tag="y")
                    for ch in range(2):
                        nc.tensor.matmul(
                            yt[:],
                            wpj[:, ch, 128 * cc : 128 * (cc + 1)],
                            xoutT[:, ch, :],
                            start=(ch == 0), stop=(ch == 1),
                        )
                    ys = ysb.tile([128, F], bf16, tag="ys")
                    with nc.allow_low_precision("y store bf16"):
                        nc.vector.tensor_copy(ys[:], yt[:])
                    nc.gpsimd.dma_start(
                        out=y_ext[cc, :, n0 : n0 + F], in_=ys[:]
                    )

            # software pipeline: PE order per iteration is
            #   [spad'/attnV g-1] [logits g] [proj g-2]
            # and the scalar queue sees exps(g) before recip(g-1), so the
            # next group's exps are never stuck behind the normalizer.
            prev_pt = phase_a(0)
            prev_xo = None
            for g in range(1, NGROUPS):
                sb2, Xts = phase_b1_mm(g - 1, prev_pt)
                cur_pt = phase_a(g)
                cur_xo = phase_b1_fix(g - 1, sb2, Xts)
                if prev_xo is not None:
                    phase_b2(g - 2, prev_xo)
                prev_pt, prev_xo = cur_pt, cur_xo
            sb2, Xts = phase_b1_mm(NGROUPS - 1, prev_pt)
            cur_xo = phase_b1_fix(NGROUPS - 1, sb2, Xts)
            phase_b2(NGROUPS - 2, prev_xo)
            phase_b2(NGROUPS - 1, cur_xo)
    return nc


# --------------------------------------------------------------------------
# Host orchestration
# --------------------------------------------------------------------------
_GLOBAL_SCALES = [2.0 ** 12, 2.0 ** 13]  # (1/Sk placeholder, Sv) - reset below


def _pow2_scale(absmax, target=120.0):
    return float(2.0 ** np.floor(np.log2(target / max(absmax, 1e-30))))


def kernel(
    voxel_features,
    cluster_assignments,
    w_qkv,
    w_proj,
    b_proj,
    cluster_bias,
):
    _install_fixup()
    x_all = np.ascontiguousarray(np.asarray(voxel_features, dtype=np.float32))
    A_all = np.ascontiguousarray(np.asarray(cluster_assignments, dtype=np.float32))
    w_qkv = np.asarray(w_qkv, dtype=np.float32)
    w_proj_np = np.ascontiguousarray(np.asarray(w_proj, dtype=np.float32))
    b_proj_np = np.asarray(b_proj, dtype=np.float32)
    cb = np.asarray(cluster_bias, dtype=np.float32)

    W_q = w_qkv[:, 0:C]
    W_k = w_qkv[:, C : 2 * C]
    W_v = w_qkv[:, 2 * C : 3 * C]

    trace = bool(os.environ.get("BASS_PROFILE"))
    if trace:
        _install_profhook()
    global LAST_EXEC_NS, TRACE_DIRS
    TRACE_DIRS = []

    # ---------------- pass 1 ----------------
    nc1 = _build_pass1()
    in_maps1 = []
    for core in range(NCORES):
        b, half = core // 2, core % 2
        xl = x_all[b, half * NLOC : (half + 1) * NLOC]
        al = A_all[b, half * NLOC : (half + 1) * NLOC]
        in_maps1.append(
            {
                "x": np.ascontiguousarray(xl.reshape(128, 64, C).astype(BF16)),
                "a": np.ascontiguousarray(al.reshape(128, 64, K).astype(BF16)),
            }
        )
    kw1 = {}
    if trace:
        import tempfile
        d = tempfile.mkdtemp(prefix="p1_trace_")
        TRACE_DIRS.append(d)
        kw1 = dict(trace=True, tmpdir=d)
    res1 = run_bass_kernel_spmd(nc1, in_maps1, list(range(NCORES)), **kw1)
    exec1 = getattr(res1, "exec_time_ns", None)
    xp_parts = np.stack([res1.results[c]["xp"] for c in range(NCORES)])  # (8,64,256)

    # ---------------- host glue ----------------
    denom = A_all.sum(axis=1) + EPS  # (B, K)

    Wq3 = W_q.reshape(C, H, D)
    Wkcl_all, v3_all = [], []
    for b in range(B):
        xp = xp_parts[2 * b] + xp_parts[2 * b + 1]
        pooled = xp / denom[b][:, None]
        k_cl = pooled @ W_k
        v_cl = pooled @ W_v
        k3 = k_cl.reshape(K, H, D)
        Wkcl = np.einsum("chd,khd->chk", Wq3, k3).reshape(C, H * K) * SCALE
        Wkcl_all.append(Wkcl)
        v3_all.append(v_cl.reshape(K, H, D))

    Sk = _pow2_scale(max(np.abs(w).max() for w in Wkcl_all))
    Sv = _pow2_scale(max(np.abs(v).max() for v in v3_all))
    _GLOBAL_SCALES[0] = 1.0 / Sk
    _GLOBAL_SCALES[1] = Sv

    # constants shared by all cores
    cb2 = np.zeros((K, 128), np.float32)
    cb2[:, 0:64] = cb * Sk
    cb2[:, 64:128] = cb * Sk
    cb2_bf = cb2.astype(BF16)

    # wide denominator indicator: s lands pre-broadcast in attn-out layout.
    # sind2[p, j, m] = 1 iff 2*j + p//64 == m//32 (same for both P tiles).
    sind2 = np.zeros((128, 2, 128), np.float32)
    for p in range(128):
        for j in range(2):
            blk = 2 * j + p // 64
            sind2[p, j, 32 * blk : 32 * blk + 32] = 1.0
    sind2_8 = sind2.astype(E4)

    wpj = np.ascontiguousarray(
        w_proj_np.reshape(2, 128, C).transpose(1, 0, 2)
    ).astype(BF16)  # [p, ch, c]

    wkcl8_all, vbd8_all = [], []
    for b in range(B):
        # [p, m, ch, mcol]: chunk-m weight blocks with the 2 c-halves adjacent
        wk = (Wkcl_all[b] * Sk).reshape(2, 128, 4, 128).transpose(1, 2, 0, 3)
        wkcl8_all.append(np.ascontiguousarray(wk).astype(E4))
        v3s = v3_all[b] * Sv
        vbd = np.zeros((128, 2, 2, 128), np.float32)
        for t in range(2):
            for jj in range(2):
                for h2 in range(2):
                    c0 = jj * 64 + 32 * h2
                    vbd[64 * h2 : 64 * (h2 + 1), t, jj, c0 : c0 + 32] = v3s[
                        :, 4 * t + 2 * jj + h2, :
                    ]
        vbd8_all.append(vbd.astype(E4))

    # ---------------- pass 2 ----------------
    nc2 = _build_pass2(float(1.0 / Sk))
    in_maps2 = []
    for core in range(NCORES):
        b, half = core // 2, core % 2
        xl = x_all[b, half * NLOC : (half + 1) * NLOC]
        al = A_all[b, half * NLOC : (half + 1) * NLOC]
        in_maps2.append(
            {
                # [mg, p, gi, ch, nf]: per 512-col group, the 2 c-half
                # k-tiles adjacent in the free dim (DoubleRow layout)
                "xs": np.ascontiguousarray(
                    xl.T.reshape(2, 128, NMG, MG // F, F).transpose(2, 1, 3, 0, 4)
                ).astype(E4),
                "as_": np.ascontiguousarray(al.T).astype(BF16),
                "wkcl": wkcl8_all[b],
                "cb2": cb2_bf,
                "vbd": vbd8_all[b],
                "sind2": sind2_8,
                "wpj": wpj,
            }
        )
    kw2 = {}
    if trace:
        import tempfile
        d = tempfile.mkdtemp(prefix="p2_trace_")
        TRACE_DIRS.append(d)
        kw2 = dict(trace=True, tmpdir=d)
    res2 = run_bass_kernel_spmd(nc2, in_maps2, list(range(NCORES)), **kw2)
    exec2 = getattr(res2, "exec_time_ns", None)
    if exec1 is not None and exec2 is not None:
        LAST_EXEC_NS = exec1 + exec2
        globals()["LAST_EXEC_SPLIT"] = (exec1, exec2)

    inv_sv = 1.0 / Sv
    y_out = np.zeros((B, N, C), np.float32)
    for core in range(NCORES):
        b, half = core // 2, core % 2
        yv = res2.results[core]["y"].astype(np.float32)  # (2,128,NLOC), *Sv
        y_out[b, half * NLOC : (half + 1) * NLOC] = (
            yv.transpose(2, 0, 1).reshape(NLOC, C) * inv_sv
        )
    y_out += b_proj_np[None, None, :]
    return y_out



# revision 24
# speedup vs baseline: 1.0555x; 1.0187x over previous
"""ClusterAwareAttention Trainium2 kernel (8 NeuronCores, axon/PJRT path).

Sharding: data-parallel over (batch, sequence-half) -> 8 shards of 8192 rows.

Two launches:
  Pass 1: per-shard cluster pooling partial sums  xp = A_loc^T @ x_loc  (64, 256)
          in bf16 (p-major contiguous layout for full-rate DMA).
  Host:   reduce halves, build pooled K/V-derived constants:
            Wk_cl (x @ Wk_cl = q @ k_cluster^T * scale, folded through W_q),
            VBD block-diagonal v_cluster, cluster-bias, indicators; global
            pow2 scales Sk (fp8 Wk_cl) and Sv (fp8 v_cluster).
  Pass 2: per-shard fused attention, cluster-major (n on the free axis,
          512-col groups):
            fp8 DoubleRow logits (256-deep contraction in one stream)
            + bf16 cluster bias -> exp (fp8 out, scale=1/Sk)
            -> fp8 DoubleRow denominators -> DVE reciprocal (bf16)
            -> bf16 broadcast matmul -> fp8 DoubleRow attn@V
            -> DVE normalize (bf16) -> bf16 output projection
            -> direct PSUM->DRAM y writes (f32; bias + 1/Sv applied on host).

Precision (validated on host): pooling bf16, logits fp8 (scaled), P fp8,
v_cluster fp8 (scaled), everything else bf16/f32 accum. rel err ~8e-3.
"""

import json
import os
from functools import lru_cache

import numpy as np

import concourse.bass as bass
import concourse.tile as tile
from concourse import mybir
from concourse.bass_utils import run_bass_kernel_spmd

import ml_dtypes

BF16 = ml_dtypes.bfloat16
E4 = ml_dtypes.float8_e4m3   # mybir.dt.float8e4 <-> jnp/ml_dtypes float8_e4m3 (max 240)

B, N, C, H, K = 4, 16384, 256, 8, 64
D = C // H
EPS = 1e-8
SCALE = D ** -0.5
NLOC = N // 2           # rows per shard
F = 512                 # group size (n on the free axis)
NGROUPS = NLOC // F     # 16
MG = 2048               # DMA macro-group (4 groups per input DMA chunk)
NMG = NLOC // MG        # 4
NCORES = 8

f32 = mybir.dt.float32
bf16 = mybir.dt.bfloat16
fp8 = mybir.dt.float8e4
DR = mybir.MatmulPerfMode.DoubleRow


# --------------------------------------------------------------------------
# BIR fixup: this container's walrus rejects instructions with >1 sync wait.
# Split extra waits onto single-wait EventSemaphore instructions just before.
# --------------------------------------------------------------------------
def _split_block(bb, counter):
    insts = bb.get("instructions")
    if insts:
        new_insts = []
        for inst in insts:
            si = inst.get("sync_info") or {}
            waits = si.get("on_wait") or []
            if len(waits) > 1:
                for w in waits[:-1]:
                    counter[0] += 1
                    new_insts.append(
                        {
                            "debug": inst.get("debug", 0),
                            "engine": inst["engine"],
                            "ins": [],
                            "name": f"WSPLIT-{counter[0]}",
                            "opcode": "EventSemaphore",
                            "outs": [],
                            "sync_info": {"on_update": [], "on_wait": [w]},
                        }
                    )
                si = dict(si)
                si["on_wait"] = [waits[-1]]
                inst = dict(inst)
                inst["sync_info"] = si
            new_insts.append(inst)
        bb["instructions"] = new_insts
    for sub in bb.get("blocks", []) or []:
        _split_block(sub, counter)


def _fixup_bir_json(bir_json: bytes) -> bytes:
    bir = json.loads(bir_json)
    counter = [0]
    for fn in bir.get("functions", []):
        for bb in fn.get("blocks", []) or []:
            _split_block(bb, counter)
    return json.dumps(bir).encode()


LAST_EXEC_NS = None
TRACE_DIRS = []


def _scalar_recip(nc, out, in_):
    """Raw ACT Reciprocal (bass blocks the wrapper for accuracy reasons;
    our 1/s operands are well-conditioned and the rel-err gate validates)."""
    se = nc.scalar
    ins = [se.lower_ap(in_)]
    for v in (0.0, 1.0, 0.0):  # bias, scale, alpha
        ins.append(mybir.ImmediateValue(dtype=mybir.dt.float32, value=v))
    return se.add_instruction(
        mybir.InstActivation(
            name=nc.get_next_instruction_name(),
            func=mybir.ActivationFunctionType.Reciprocal,
            ins=ins,
            outs=[se.lower_ap(out)],
        )
    )


def _install_profhook():
    import sys
    import types

    if "antenv.axon_hooks" in sys.modules:
        return
    import antenv

    mod = types.ModuleType("antenv.axon_hooks")
    _hook = [None]
    mod.set_axon_ntff_profile_hook = lambda h: _hook.__setitem__(0, h)
    mod.get_axon_ntff_profile_hook = lambda: _hook[0]
    sys.modules["antenv.axon_hooks"] = mod
    antenv.axon_hooks = mod
    from trn_agent_boot.trn_boot import _ntff_profile_via_ctypes

    mod.set_axon_ntff_profile_hook(
        _ntff_profile_via_ctypes("/opt/axon/libaxon_pjrt.so")
    )


_fixup_installed = False


def _install_fixup():
    global _fixup_installed
    if _fixup_installed:
        return
    _fixup_installed = True
    import concourse.bass_utils as bu
    import concourse.bass2jax as b2j

    orig = bu.compile_bir_kernel

    def patched(bir_json, tmpdir, neff_name="file.neff"):
        return orig(_fixup_bir_json(bir_json), tmpdir, neff_name=neff_name)

    bu.compile_bir_kernel = patched
    b2j.compile_bir_kernel = patched

    if os.environ.get("BASS_LDW_OPT"):
        orig_run = bu.run_command

        def patched_run(cmd, *a, **kw):
            cmd = [
                c.replace("--enable-ldw-opt=false", "--enable-ldw-opt=true")
                if isinstance(c, str)
                else c
                for c in cmd
            ]
            return orig_run(cmd, *a, **kw)

        bu.run_command = patched_run


# --------------------------------------------------------------------------
# Pass 1: xp[k, c] = sum_n A_loc[n, k] * x_loc[n, c], bf16 in / f32 accum.
# p-major layout: row n = 64*p + i  ->  x[p, i, c] contiguous per partition.
# --------------------------------------------------------------------------
@lru_cache(maxsize=1)
def _build_pass1():
    nc = bass.Bass()
    x_ext = nc.declare_dram_parameter("x", [128, 64, C], bf16, isOutput=False)
    a_ext = nc.declare_dram_parameter("a", [128, 64, K], bf16, isOutput=False)
    xp_ext = nc.declare_dram_parameter("xp", [K, C], f32, isOutput=True)

    XCH = 8     # i's per x DMA chunk (128, 8, 256) bf16 = 4KB/partition
    ACH = 8     # i's per a DMA chunk (128, 8, 64) bf16 = 1KB/partition

    with tile.TileContext(nc) as tc:
        with (
            tc.tile_pool(name="xin", bufs=64 // XCH) as xin,
            tc.tile_pool(name="ain", bufs=64 // ACH) as ain,
            tc.tile_pool(name="acc", bufs=1, space="PSUM") as accp,
            tc.tile_pool(name="outp", bufs=1) as outp,
        ):
            qs = [nc.sync, nc.scalar, nc.gpsimd]
            # interleave a/x chunk loads so matmul i can start as soon as
            # chunk i//8 of BOTH streams has landed; a-chunks (small) ride
            # the hw queues, x-chunks (big) round-robin all three
            a_sb, x_sb = [], []
            for ci in range(64 // XCH):
                ag = ain.tile([128, ACH, K], bf16, tag="a")
                qs[ci % 2].dma_start(
                    out=ag[:], in_=a_ext[:, ci * ACH : (ci + 1) * ACH, :]
                )
                a_sb.append(ag)
                xg = xin.tile([128, XCH, C], bf16, tag="x")
                qs[ci % 3].dma_start(
                    out=xg[:], in_=x_ext[:, ci * XCH : (ci + 1) * XCH, :]
                )
                x_sb.append(xg)

            acc = accp.tile([K, C], f32)
            for i in range(64):
                nc.tensor.matmul(
                    acc[:],
                    a_sb[i // ACH][:, i % ACH, :],
                    x_sb[i // XCH][:, i % XCH, :],
                    start=(i == 0),
                    stop=(i == 63),
                )
            xps = outp.tile([K, C], f32)
            nc.vector.tensor_copy(xps[:], acc[:])
            nc.sync.dma_start(out=xp_ext[:], in_=xps[:])
    return nc


# --------------------------------------------------------------------------
# Pass 2: full attention for one shard, cluster-major.
# v2: denominator sums computed pre-broadcast (wide indicator lhsT), DVE
# fast-reciprocal for 1/s, one-group software-pipeline skew on the PE.
# --------------------------------------------------------------------------
@lru_cache(maxsize=1)
def _build_pass2(sk_inv: float):
    nc = bass.Bass()
    xs_ext = nc.declare_dram_parameter(
        "xs", [NMG, 128, MG // F, 2, F], fp8, isOutput=False
    )
    as_ext = nc.declare_dram_parameter("as_", [K, NLOC], bf16, isOutput=False)
    wkcl_ext = nc.declare_dram_parameter("wkcl", [128, 4, 2, 128], fp8, isOutput=False)
    cb2_ext = nc.declare_dram_parameter("cb2", [K, 128], bf16, isOutput=False)
    vbd_ext = nc.declare_dram_parameter("vbd", [128, 2, 2, 128], fp8, isOutput=False)
    sind2_ext = nc.declare_dram_parameter("sind2", [128, 2, 128], fp8, isOutput=False)
    wpj_ext = nc.declare_dram_parameter("wpj", [128, 2, C], bf16, isOutput=False)
    y_ext = nc.declare_dram_parameter("y", [2, 128, NLOC], bf16, isOutput=True)

    with tile.TileContext(nc) as tc:
        with (
            tc.tile_pool(name="const", bufs=1) as const,
            tc.tile_pool(name="xt", bufs=NMG) as xtp,
            tc.tile_pool(name="at", bufs=NMG) as atp,
            tc.tile_pool(name="lgp", bufs=2, space="PSUM") as lgp,
            tc.tile_pool(name="spp", bufs=1, space="PSUM") as spp,
            tc.tile_pool(name="xtp2", bufs=2, space="PSUM") as xtp2,
            tc.tile_pool(name="ypp", bufs=2, space="PSUM") as ypp,
            tc.tile_pool(name="pp", bufs=3) as pp,
            tc.tile_pool(name="rp", bufs=4) as rp,
            tc.tile_pool(name="xon", bufs=3) as xon,
            tc.tile_pool(name="ysb", bufs=3) as ysb,
        ):
            wkcl = const.tile([128, 4, 2, 128], fp8)
            nc.sync.dma_start(out=wkcl[:], in_=wkcl_ext[:])
            cb2 = const.tile([K, 128], bf16)
            nc.sync.dma_start(out=cb2[:], in_=cb2_ext[:])

            # inputs split per-gi: each group's slice is one DMA, so every
            # logits matmul waits on exactly one transfer
            qrot = [nc.sync, nc.scalar, nc.gpsimd]
            xT_all, aT_all = [], []
            for mg in range(NMG):
                n0 = mg * MG
                xT = xtp.tile([128, MG // F, 2, F], fp8, tag="xT")
                for gi in range(4):
                    qrot[(4 * mg + gi) % 3].dma_start(
                        out=xT[:, gi : gi + 1, :, :],
                        in_=xs_ext[mg, :, gi : gi + 1, :, :],
                    )
                aT = atp.tile([K, MG], bf16, tag="aT")
                qrot[(4 * mg) % 3].dma_start(
                    out=aT[:], in_=as_ext[:, n0 : n0 + MG]
                )
                xT_all.append(xT)
                aT_all.append(aT)
                if mg == 0:
                    vbd = const.tile([128, 2, 2, 128], fp8)
                    nc.gpsimd.dma_start(out=vbd[:], in_=vbd_ext[:])
                    sind2 = const.tile([128, 2, 128], fp8)
                    nc.gpsimd.dma_start(out=sind2[:], in_=sind2_ext[:])
                    wpj = const.tile([128, 2, C], bf16)
                    nc.gpsimd.dma_start(out=wpj[:], in_=wpj_ext[:])

            # phase A: logits chunks (+ cluster bias) -> exp -> Pt (fp8)
            def phase_a(g):
                mg, gi, off = g // 4, g % 4, (g % 4) * F
                xT, aT = xT_all[mg], aT_all[mg]
                Pt4 = pp.tile([128, 4, F], fp8, tag="P", name=f"Pt_{g}")
                for m in range(4):
                    lg = lgp.tile([128, F], f32, tag="lg")
                    nc.tensor.matmul(
                        lg[:],
                        wkcl[:, m, :, :],
                        xT[:, gi, :, :],
                        start=True, stop=False,
                        perf_mode=DR,
                    )
                    nc.tensor.matmul(
                        lg[:], cb2[:], aT[:, off : off + F],
                        start=False, stop=True,
                    )
                    nc.scalar.activation(
                        Pt4[:, m, :], lg[:],
                        mybir.ActivationFunctionType.Exp, scale=sk_inv,
                    )
                return [Pt4[:, 0:2, :], Pt4[:, 2:4, :]]

            # phase B1a: denominator + attn@V matmuls (PE only)
            def phase_b1_mm(g, Pt):
                sb2 = spp.tile([128, 2, F], f32, tag="sb")
                for t in range(2):
                    nc.tensor.matmul(
                        sb2[:, t, :], sind2[:], Pt[t][:],
                        start=True, stop=True,
                        perf_mode=DR,
                    )
                Xts = []
                for t in range(2):
                    Xt = xtp2.tile([128, F], f32, tag="Xt")
                    nc.tensor.matmul(
                        Xt[:], vbd[:, t, :, :], Pt[t][:],
                        start=True, stop=True,
                        perf_mode=DR,
                    )
                    Xts.append(Xt)
                return sb2, Xts

            # phase B1b: 1/s (already broadcast) + normalize (scalar + DVE)
            # (ln+exp, NOT Reciprocal: exp/ln/copy share one ACT table set;
            # reciprocal lives in another and thrashes ACT_TABLE_LOAD)
            def phase_b1_fix(g, sb2, Xts):
                lns = rp.tile([128, 2, F], f32, tag="lns")
                nc.scalar.activation(
                    lns[:], sb2[:], mybir.ActivationFunctionType.Ln
                )
                rb = rp.tile([128, 2, F], f32, tag="rb")
                nc.scalar.activation(
                    rb[:], lns[:], mybir.ActivationFunctionType.Exp,
                    scale=-1.0,
                )
                xoutT = xon.tile([128, 2, F], bf16)
                for t in range(2):
                    with nc.allow_low_precision("normalized attn out bf16"):
                        nc.vector.tensor_mul(
                            xoutT[:, t, :], Xts[t][:], rb[:, t, :]
                        )
                return xoutT

            # phase B2: output projection -> y store
            def phase_b2(g, xoutT):
                n0 = g * F
                for cc in range(2):
                    yt = ypp.tile([128, F], f32, tag="y")
                    for ch in range(2):
                        nc.tensor.matmul(
                            yt[:],
                            wpj[:, ch, 128 * cc : 128 * (cc + 1)],
                            xoutT[:, ch, :],
                            start=(ch == 0), stop=(ch == 1),
                        )
                    ys = ysb.tile([128, F], bf16, tag="ys")
                    with nc.allow_low_precision("y store bf16"):
                        nc.vector.tensor_copy(ys[:], yt[:])
                    (nc.gpsimd if cc == 0 else nc.sync).dma_start(
                        out=y_ext[cc, :, n0 : n0 + F], in_=ys[:]
                    )

            # software pipeline: PE order per iteration is
            #   [spad'/attnV g-1] [logits g] [proj g-2]
            # and the scalar queue sees exps(g) before recip(g-1), so the
            # next group's exps are never stuck behind the normalizer.
            prev_pt = phase_a(0)
            prev_xo = None
            for g in range(1, NGROUPS):
                sb2, Xts = phase_b1_mm(g - 1, prev_pt)
                cur_pt = phase_a(g)
                cur_xo = phase_b1_fix(g - 1, sb2, Xts)
                if prev_xo is not None:
                    phase_b2(g - 2, prev_xo)
                prev_pt, prev_xo = cur_pt, cur_xo
            sb2, Xts = phase_b1_mm(NGROUPS - 1, prev_pt)
            cur_xo = phase_b1_fix(NGROUPS - 1, sb2, Xts)
            phase_b2(NGROUPS - 2, prev_xo)
            phase_b2(NGROUPS - 1, cur_xo)
    return nc


# --------------------------------------------------------------------------
# Host orchestration
# --------------------------------------------------------------------------
_GLOBAL_SCALES = [2.0 ** 12, 2.0 ** 13]  # (1/Sk placeholder, Sv) - reset below


def _pow2_scale(absmax, target=120.0):
    return float(2.0 ** np.floor(np.log2(target / max(absmax, 1e-30))))


def kernel(
    voxel_features,
    cluster_assignments,
    w_qkv,
    w_proj,
    b_proj,
    cluster_bias,
):
    _install_fixup()
    x_all = np.ascontiguousarray(np.asarray(voxel_features, dtype=np.float32))
    A_all = np.ascontiguousarray(np.asarray(cluster_assignments, dtype=np.float32))
    w_qkv = np.asarray(w_qkv, dtype=np.float32)
    w_proj_np = np.ascontiguousarray(np.asarray(w_proj, dtype=np.float32))
    b_proj_np = np.asarray(b_proj, dtype=np.float32)
    cb = np.asarray(cluster_bias, dtype=np.float32)

    W_q = w_qkv[:, 0:C]
    W_k = w_qkv[:, C : 2 * C]
    W_v = w_qkv[:, 2 * C : 3 * C]

    trace = bool(os.environ.get("BASS_PROFILE"))
    if trace:
        _install_profhook()
    global LAST_EXEC_NS, TRACE_DIRS
    TRACE_DIRS = []

    # ---------------- pass 1 ----------------
    nc1 = _build_pass1()
    in_maps1 = []
    for core in range(NCORES):
        b, half = core // 2, core % 2
        xl = x_all[b, half * NLOC : (half + 1) * NLOC]
        al = A_all[b, half * NLOC : (half + 1) * NLOC]
        in_maps1.append(
            {
                "x": np.ascontiguousarray(xl.reshape(128, 64, C).astype(BF16)),
                "a": np.ascontiguousarray(al.reshape(128, 64, K).astype(BF16)),
            }
        )
    kw1 = {}
    if trace:
        import tempfile
        d = tempfile.mkdtemp(prefix="p1_trace_")
        TRACE_DIRS.append(d)
        kw1 = dict(trace=True, tmpdir=d)
    res1 = run_bass_kernel_spmd(nc1, in_maps1, list(range(NCORES)), **kw1)
    exec1 = getattr(res1, "exec_time_ns", None)
    xp_parts = np.stack([res1.results[c]["xp"] for c in range(NCORES)])  # (8,64,256)

    # ---------------- host glue ----------------
    denom = A_all.sum(axis=1) + EPS  # (B, K)

    Wq3 = W_q.reshape(C, H, D)
    Wkcl_all, v3_all = [], []
    for b in range(B):
        xp = xp_parts[2 * b] + xp_parts[2 * b + 1]
        pooled = xp / denom[b][:, None]
        k_cl = pooled @ W_k
        v_cl = pooled @ W_v
        k3 = k_cl.reshape(K, H, D)
        Wkcl = np.einsum("chd,khd->chk", Wq3, k3).reshape(C, H * K) * SCALE
        Wkcl_all.append(Wkcl)
        v3_all.append(v_cl.reshape(K, H, D))

    Sk = _pow2_scale(max(np.abs(w).max() for w in Wkcl_all))
    Sv = _pow2_scale(max(np.abs(v).max() for v in v3_all))
    _GLOBAL_SCALES[0] = 1.0 / Sk
    _GLOBAL_SCALES[1] = Sv

    # constants shared by all cores
    cb2 = np.zeros((K, 128), np.float32)
    cb2[:, 0:64] = cb * Sk
    cb2[:, 64:128] = cb * Sk
    cb2_bf = cb2.astype(BF16)

    # wide denominator indicator: s lands pre-broadcast in attn-out layout.
    # sind2[p, j, m] = 1 iff 2*j + p//64 == m//32 (same for both P tiles).
    sind2 = np.zeros((128, 2, 128), np.float32)
    for p in range(128):
        for j in range(2):
            blk = 2 * j + p // 64
            sind2[p, j, 32 * blk : 32 * blk + 32] = 1.0
    sind2_8 = sind2.astype(E4)

    wpj = np.ascontiguousarray(
        w_proj_np.reshape(2, 128, C).transpose(1, 0, 2)
    ).astype(BF16)  # [p, ch, c]

    wkcl8_all, vbd8_all = [], []
    for b in range(B):
        # [p, m, ch, mcol]: chunk-m weight blocks with the 2 c-halves adjacent
        wk = (Wkcl_all[b] * Sk).reshape(2, 128, 4, 128).transpose(1, 2, 0, 3)
        wkcl8_all.append(np.ascontiguousarray(wk).astype(E4))
        v3s = v3_all[b] * Sv
        vbd = np.zeros((128, 2, 2, 128), np.float32)
        for t in range(2):
            for jj in range(2):
                for h2 in range(2):
                    c0 = jj * 64 + 32 * h2
                    vbd[64 * h2 : 64 * (h2 + 1), t, jj, c0 : c0 + 32] = v3s[
                        :, 4 * t + 2 * jj + h2, :
                    ]
        vbd8_all.append(vbd.astype(E4))

    # ---------------- pass 2 ----------------
    nc2 = _build_pass2(float(1.0 / Sk))
    in_maps2 = []
    for core in range(NCORES):
        b, half = core // 2, core % 2
        xl = x_all[b, half * NLOC : (half + 1) * NLOC]
        al = A_all[b, half * NLOC : (half + 1) * NLOC]
        in_maps2.append(
            {
                # [mg, p, gi, ch, nf]: per 512-col group, the 2 c-half
                # k-tiles adjacent in the free dim (DoubleRow layout)
                "xs": np.ascontiguousarray(
                    xl.T.reshape(2, 128, NMG, MG // F, F).transpose(2, 1, 3, 0, 4)
                ).astype(E4),
                "as_": np.ascontiguousarray(al.T).astype(BF16),
                "wkcl": wkcl8_all[b],
                "cb2": cb2_bf,
                "vbd": vbd8_all[b],
                "sind2": sind2_8,
                "wpj": wpj,
            }
        )
    kw2 = {}
    if trace:
        import tempfile
        d = tempfile.mkdtemp(prefix="p2_trace_")
        TRACE_DIRS.append(d)
        kw2 = dict(trace=True, tmpdir=d)
    res2 = run_bass_kernel_spmd(nc2, in_maps2, list(range(NCORES)), **kw2)
    exec2 = getattr(res2, "exec_time_ns", None)
    if exec1 is not None and exec2 is not None:
        LAST_EXEC_NS = exec1 + exec2
        globals()["LAST_EXEC_SPLIT"] = (exec1, exec2)

    inv_sv = 1.0 / Sv
    y_out = np.zeros((B, N, C), np.float32)
    for core in range(NCORES):
        b, half = core // 2, core % 2
        yv = res2.results[core]["y"].astype(np.float32)  # (2,128,NLOC), *Sv
        y_out[b, half * NLOC : (half + 1) * NLOC] = (
            yv.transpose(2, 0, 1).reshape(NLOC, C) * inv_sv
        )
    y_out += b_proj_np[None, None, :]
    return y_out



# revision 26
# speedup vs baseline: 1.0653x; 1.0092x over previous
"""ClusterAwareAttention Trainium2 kernel (8 NeuronCores, axon/PJRT path).

Sharding: data-parallel over (batch, sequence-half) -> 8 shards of 8192 rows.

Two launches:
  Pass 1: per-shard cluster pooling partial sums  xp = A_loc^T @ x_loc  (64, 256)
          in bf16 (p-major contiguous layout for full-rate DMA).
  Host:   reduce halves, build pooled K/V-derived constants:
            Wk_cl (x @ Wk_cl = q @ k_cluster^T * scale, folded through W_q),
            VBD block-diagonal v_cluster, cluster-bias, indicators; global
            pow2 scales Sk (fp8 Wk_cl) and Sv (fp8 v_cluster).
  Pass 2: per-shard fused attention, cluster-major (n on the free axis,
          512-col groups):
            fp8 DoubleRow logits (256-deep contraction in one stream)
            + bf16 cluster bias -> exp (fp8 out, scale=1/Sk)
            -> fp8 DoubleRow denominators -> DVE reciprocal (bf16)
            -> bf16 broadcast matmul -> fp8 DoubleRow attn@V
            -> DVE normalize (bf16) -> bf16 output projection
            -> direct PSUM->DRAM y writes (f32; bias + 1/Sv applied on host).

Precision (validated on host): pooling bf16, logits fp8 (scaled), P fp8,
v_cluster fp8 (scaled), everything else bf16/f32 accum. rel err ~8e-3.
"""

import json
import os
from functools import lru_cache

import numpy as np

import concourse.bass as bass
import concourse.tile as tile
from concourse import mybir
from concourse.bass_utils import run_bass_kernel_spmd

import ml_dtypes

BF16 = ml_dtypes.bfloat16
E4 = ml_dtypes.float8_e4m3   # mybir.dt.float8e4 <-> jnp/ml_dtypes float8_e4m3 (max 240)

B, N, C, H, K = 4, 16384, 256, 8, 64
D = C // H
EPS = 1e-8
SCALE = D ** -0.5
NLOC = N // 2           # rows per shard
F = 512                 # group size (n on the free axis)
NGROUPS = NLOC // F     # 16
MG = 2048               # DMA macro-group (4 groups per input DMA chunk)
NMG = NLOC // MG        # 4
NCORES = 8

f32 = mybir.dt.float32
bf16 = mybir.dt.bfloat16
fp8 = mybir.dt.float8e4
DR = mybir.MatmulPerfMode.DoubleRow


# --------------------------------------------------------------------------
# BIR fixup: this container's walrus rejects instructions with >1 sync wait.
# Split extra waits onto single-wait EventSemaphore instructions just before.
# --------------------------------------------------------------------------
def _split_block(bb, counter):
    insts = bb.get("instructions")
    if insts:
        new_insts = []
        for inst in insts:
            si = inst.get("sync_info") or {}
            waits = si.get("on_wait") or []
            if len(waits) > 1:
                for w in waits[:-1]:
                    counter[0] += 1
                    new_insts.append(
                        {
                            "debug": inst.get("debug", 0),
                            "engine": inst["engine"],
                            "ins": [],
                            "name": f"WSPLIT-{counter[0]}",
                            "opcode": "EventSemaphore",
                            "outs": [],
                            "sync_info": {"on_update": [], "on_wait": [w]},
                        }
                    )
                si = dict(si)
                si["on_wait"] = [waits[-1]]
                inst = dict(inst)
                inst["sync_info"] = si
            new_insts.append(inst)
        bb["instructions"] = new_insts
    for sub in bb.get("blocks", []) or []:
        _split_block(sub, counter)


def _fixup_bir_json(bir_json: bytes) -> bytes:
    bir = json.loads(bir_json)
    counter = [0]
    for fn in bir.get("functions", []):
        for bb in fn.get("blocks", []) or []:
            _split_block(bb, counter)
    return json.dumps(bir).encode()


LAST_EXEC_NS = None
TRACE_DIRS = []


def _scalar_recip(nc, out, in_):
    """Raw ACT Reciprocal (bass blocks the wrapper for accuracy reasons;
    our 1/s operands are well-conditioned and the rel-err gate validates)."""
    se = nc.scalar
    ins = [se.lower_ap(in_)]
    for v in (0.0, 1.0, 0.0):  # bias, scale, alpha
        ins.append(mybir.ImmediateValue(dtype=mybir.dt.float32, value=v))
    return se.add_instruction(
        mybir.InstActivation(
            name=nc.get_next_instruction_name(),
            func=mybir.ActivationFunctionType.Reciprocal,
            ins=ins,
            outs=[se.lower_ap(out)],
        )
    )


def _install_profhook():
    import sys
    import types

    if "antenv.axon_hooks" in sys.modules:
        return
    import antenv

    mod = types.ModuleType("antenv.axon_hooks")
    _hook = [None]
    mod.set_axon_ntff_profile_hook = lambda h: _hook.__setitem__(0, h)
    mod.get_axon_ntff_profile_hook = lambda: _hook[0]
    sys.modules["antenv.axon_hooks"] = mod
    antenv.axon_hooks = mod
    from trn_agent_boot.trn_boot import _ntff_profile_via_ctypes

    mod.set_axon_ntff_profile_hook(
        _ntff_profile_via_ctypes("/opt/axon/libaxon_pjrt.so")
    )


_fixup_installed = False


def _install_fixup():
    global _fixup_installed
    if _fixup_installed:
        return
    _fixup_installed = True
    import concourse.bass_utils as bu
    import concourse.bass2jax as b2j

    orig = bu.compile_bir_kernel

    def patched(bir_json, tmpdir, neff_name="file.neff"):
        return orig(_fixup_bir_json(bir_json), tmpdir, neff_name=neff_name)

    bu.compile_bir_kernel = patched
    b2j.compile_bir_kernel = patched

    if os.environ.get("BASS_LDW_OPT"):
        orig_run = bu.run_command

        def patched_run(cmd, *a, **kw):
            cmd = [
                c.replace("--enable-ldw-opt=false", "--enable-ldw-opt=true")
                if isinstance(c, str)
                else c
                for c in cmd
            ]
            return orig_run(cmd, *a, **kw)

        bu.run_command = patched_run


# --------------------------------------------------------------------------
# Pass 1: xp[k, c] = sum_n A_loc[n, k] * x_loc[n, c], bf16 in / f32 accum.
# p-major layout: row n = 64*p + i  ->  x[p, i, c] contiguous per partition.
# --------------------------------------------------------------------------
@lru_cache(maxsize=1)
def _build_pass1():
    nc = bass.Bass()
    x_ext = nc.declare_dram_parameter("x", [128, 64, C], bf16, isOutput=False)
    a_ext = nc.declare_dram_parameter("a", [128, 64, K], bf16, isOutput=False)
    xp_ext = nc.declare_dram_parameter("xp", [K, C], f32, isOutput=True)

    XCH = 8     # i's per x DMA chunk (128, 8, 256) bf16 = 4KB/partition
    ACH = 8     # i's per a DMA chunk (128, 8, 64) bf16 = 1KB/partition

    with tile.TileContext(nc) as tc:
        with (
            tc.tile_pool(name="xin", bufs=64 // XCH) as xin,
            tc.tile_pool(name="ain", bufs=64 // ACH) as ain,
            tc.tile_pool(name="acc", bufs=1, space="PSUM") as accp,
            tc.tile_pool(name="outp", bufs=1) as outp,
        ):
            qs = [nc.sync, nc.scalar, nc.gpsimd]
            # interleave a/x chunk loads so matmul i can start as soon as
            # chunk i//8 of BOTH streams has landed; a-chunks (small) ride
            # the hw queues, x-chunks (big) round-robin all three
            a_sb, x_sb = [], []
            for ci in range(64 // XCH):
                ag = ain.tile([128, ACH, K], bf16, tag="a")
                qs[ci % 2].dma_start(
                    out=ag[:], in_=a_ext[:, ci * ACH : (ci + 1) * ACH, :]
                )
                a_sb.append(ag)
                xg = xin.tile([128, XCH, C], bf16, tag="x")
                qs[ci % 3].dma_start(
                    out=xg[:], in_=x_ext[:, ci * XCH : (ci + 1) * XCH, :]
                )
                x_sb.append(xg)

            acc = accp.tile([K, C], f32)
            for i in range(64):
                nc.tensor.matmul(
                    acc[:],
                    a_sb[i // ACH][:, i % ACH, :],
                    x_sb[i // XCH][:, i % XCH, :],
                    start=(i == 0),
                    stop=(i == 63),
                )
            xps = outp.tile([K, C], f32)
            nc.vector.tensor_copy(xps[:], acc[:])
            nc.sync.dma_start(out=xp_ext[:], in_=xps[:])
    return nc


# --------------------------------------------------------------------------
# Pass 2: full attention for one shard, cluster-major.
# v2: denominator sums computed pre-broadcast (wide indicator lhsT), DVE
# fast-reciprocal for 1/s, one-group software-pipeline skew on the PE.
# --------------------------------------------------------------------------
@lru_cache(maxsize=1)
def _build_pass2(sk_inv: float):
    nc = bass.Bass()
    xs_ext = nc.declare_dram_parameter(
        "xs", [NMG, 128, MG // F, 2, F], fp8, isOutput=False
    )
    as_ext = nc.declare_dram_parameter("as_", [K, NLOC], bf16, isOutput=False)
    wkcl_ext = nc.declare_dram_parameter("wkcl", [128, 4, 2, 128], fp8, isOutput=False)
    cb2_ext = nc.declare_dram_parameter("cb2", [K, 128], bf16, isOutput=False)
    vbd_ext = nc.declare_dram_parameter("vbd", [128, 2, 2, 128], fp8, isOutput=False)
    sind2_ext = nc.declare_dram_parameter("sind2", [128, 2, 128], fp8, isOutput=False)
    wpj_ext = nc.declare_dram_parameter("wpj", [128, 2, C], bf16, isOutput=False)
    y_ext = nc.declare_dram_parameter("y", [2, 128, NLOC], bf16, isOutput=True)

    with tile.TileContext(nc) as tc:
        with (
            tc.tile_pool(name="const", bufs=1) as const,
            tc.tile_pool(name="xt", bufs=NMG) as xtp,
            tc.tile_pool(name="at", bufs=NMG) as atp,
            tc.tile_pool(name="lgp", bufs=2, space="PSUM") as lgp,
            tc.tile_pool(name="spp", bufs=1, space="PSUM") as spp,
            tc.tile_pool(name="xtp2", bufs=2, space="PSUM") as xtp2,
            tc.tile_pool(name="ypp", bufs=2, space="PSUM") as ypp,
            tc.tile_pool(name="pp", bufs=3) as pp,
            tc.tile_pool(name="rp", bufs=4) as rp,
            tc.tile_pool(name="xon", bufs=3) as xon,
            tc.tile_pool(name="ysb", bufs=3) as ysb,
        ):
            wkcl = const.tile([128, 4, 2, 128], fp8)
            nc.sync.dma_start(out=wkcl[:], in_=wkcl_ext[:])
            cb2 = const.tile([K, 128], bf16)
            nc.sync.dma_start(out=cb2[:], in_=cb2_ext[:])

            # inputs split per-gi: each group's slice is one DMA, so every
            # logits matmul waits on exactly one transfer
            qrot = [nc.sync, nc.scalar, nc.gpsimd]
            xT_all, aT_all = [], []
            for mg in range(NMG):
                n0 = mg * MG
                xT = xtp.tile([128, MG // F, 2, F], fp8, tag="xT")
                for gi in range(4):
                    qrot[(4 * mg + gi) % 3].dma_start(
                        out=xT[:, gi : gi + 1, :, :],
                        in_=xs_ext[mg, :, gi : gi + 1, :, :],
                    )
                aT = atp.tile([K, MG], bf16, tag="aT")
                qrot[(4 * mg) % 3].dma_start(
                    out=aT[:], in_=as_ext[:, n0 : n0 + MG]
                )
                xT_all.append(xT)
                aT_all.append(aT)
                if mg == 0:
                    vbd = const.tile([128, 2, 2, 128], fp8)
                    nc.gpsimd.dma_start(out=vbd[:], in_=vbd_ext[:])
                    sind2 = const.tile([128, 2, 128], fp8)
                    nc.gpsimd.dma_start(out=sind2[:], in_=sind2_ext[:])
                    wpj = const.tile([128, 2, C], bf16)
                    nc.gpsimd.dma_start(out=wpj[:], in_=wpj_ext[:])

            # phase A: logits chunks (+ cluster bias) -> exp -> Pt (fp8)
            def phase_a(g):
                mg, gi, off = g // 4, g % 4, (g % 4) * F
                xT, aT = xT_all[mg], aT_all[mg]
                Pt4 = pp.tile([128, 4, F], fp8, tag="P", name=f"Pt_{g}")
                for m in range(4):
                    lg = lgp.tile([128, F], f32, tag="lg")
                    nc.tensor.matmul(
                        lg[:],
                        wkcl[:, m, :, :],
                        xT[:, gi, :, :],
                        start=True, stop=False,
                        perf_mode=DR,
                    )
                    nc.tensor.matmul(
                        lg[:], cb2[:], aT[:, off : off + F],
                        start=False, stop=True,
                    )
                    nc.scalar.activation(
                        Pt4[:, m, :], lg[:],
                        mybir.ActivationFunctionType.Exp, scale=sk_inv,
                    )
                return [Pt4[:, 0:2, :], Pt4[:, 2:4, :]]

            # phase B1a: denominator + attn@V matmuls (PE only)
            def phase_b1_mm(g, Pt):
                sb2 = spp.tile([128, 2, F], f32, tag="sb")
                for t in range(2):
                    nc.tensor.matmul(
                        sb2[:, t, :], sind2[:], Pt[t][:],
                        start=True, stop=True,
                        perf_mode=DR,
                    )
                Xts = []
                for t in range(2):
                    Xt = xtp2.tile([128, F], f32, tag="Xt")
                    nc.tensor.matmul(
                        Xt[:], vbd[:, t, :, :], Pt[t][:],
                        start=True, stop=True,
                        perf_mode=DR,
                    )
                    Xts.append(Xt)
                return sb2, Xts

            # phase B1b is split so the scalar queue never blocks the next
            # group's exps: ln(g-1) is emitted BEFORE phase_a(g) (its input
            # is ready at iteration start), exp_rb + muls after.
            # (ln+exp, NOT Reciprocal: exp/ln/copy share one ACT table set;
            # reciprocal lives in another and thrashes ACT_TABLE_LOAD)
            def phase_b1_ln(g, sb2):
                lns = rp.tile([128, 2, F], f32, tag="lns")
                nc.scalar.activation(
                    lns[:], sb2[:], mybir.ActivationFunctionType.Ln
                )
                return lns

            def phase_b1_fix(g, lns, Xts):
                rb = rp.tile([128, 2, F], f32, tag="rb")
                nc.scalar.activation(
                    rb[:], lns[:], mybir.ActivationFunctionType.Exp,
                    scale=-1.0,
                )
                xoutT = xon.tile([128, 2, F], bf16)
                for t in range(2):
                    with nc.allow_low_precision("normalized attn out bf16"):
                        nc.vector.tensor_mul(
                            xoutT[:, t, :], Xts[t][:], rb[:, t, :]
                        )
                return xoutT

            # phase B2: output projection -> y store
            def phase_b2(g, xoutT):
                n0 = g * F
                for cc in range(2):
                    yt = ypp.tile([128, F], f32, tag="y")
                    for ch in range(2):
                        nc.tensor.matmul(
                            yt[:],
                            wpj[:, ch, 128 * cc : 128 * (cc + 1)],
                            xoutT[:, ch, :],
                            start=(ch == 0), stop=(ch == 1),
                        )
                    ys = ysb.tile([128, F], bf16, tag="ys")
                    with nc.allow_low_precision("y store bf16"):
                        nc.vector.tensor_copy(ys[:], yt[:])
                    (nc.gpsimd if cc == 0 else nc.sync).dma_start(
                        out=y_ext[cc, :, n0 : n0 + F], in_=ys[:]
                    )

            # software pipeline: PE order per iteration is
            #   [spad'/attnV g-1] [logits g] [proj g-2]
            # and the scalar queue sees exps(g) before recip(g-1), so the
            # next group's exps are never stuck behind the normalizer.
            prev_pt = phase_a(0)
            prev_xo = None
            for g in range(1, NGROUPS):
                sb2, Xts = phase_b1_mm(g - 1, prev_pt)
                lns = phase_b1_ln(g - 1, sb2)
                cur_pt = phase_a(g)
                cur_xo = phase_b1_fix(g - 1, lns, Xts)
                if prev_xo is not None:
                    phase_b2(g - 2, prev_xo)
                prev_pt, prev_xo = cur_pt, cur_xo
            sb2, Xts = phase_b1_mm(NGROUPS - 1, prev_pt)
            lns = phase_b1_ln(NGROUPS - 1, sb2)
            cur_xo = phase_b1_fix(NGROUPS - 1, lns, Xts)
            phase_b2(NGROUPS - 2, prev_xo)
            phase_b2(NGROUPS - 1, cur_xo)
    return nc


# --------------------------------------------------------------------------
# Host orchestration
# --------------------------------------------------------------------------
_GLOBAL_SCALES = [2.0 ** 12, 2.0 ** 13]  # (1/Sk placeholder, Sv) - reset below


def _pow2_scale(absmax, target=120.0):
    return float(2.0 ** np.floor(np.log2(target / max(absmax, 1e-30))))


def kernel(
    voxel_features,
    cluster_assignments,
    w_qkv,
    w_proj,
    b_proj,
    cluster_bias,
):
    _install_fixup()
    x_all = np.ascontiguousarray(np.asarray(voxel_features, dtype=np.float32))
    A_all = np.ascontiguousarray(np.asarray(cluster_assignments, dtype=np.float32))
    w_qkv = np.asarray(w_qkv, dtype=np.float32)
    w_proj_np = np.ascontiguousarray(np.asarray(w_proj, dtype=np.float32))
    b_proj_np = np.asarray(b_proj, dtype=np.float32)
    cb = np.asarray(cluster_bias, dtype=np.float32)

    W_q = w_qkv[:, 0:C]
    W_k = w_qkv[:, C : 2 * C]
    W_v = w_qkv[:, 2 * C : 3 * C]

    trace = bool(os.environ.get("BASS_PROFILE"))
    if trace:
        _install_profhook()
    global LAST_EXEC_NS, TRACE_DIRS
    TRACE_DIRS = []

    # ---------------- pass 1 ----------------
    nc1 = _build_pass1()
    in_maps1 = []
    for core in range(NCORES):
        b, half = core // 2, core % 2
        xl = x_all[b, half * NLOC : (half + 1) * NLOC]
        al = A_all[b, half * NLOC : (half + 1) * NLOC]
        in_maps1.append(
            {
                "x": np.ascontiguousarray(xl.reshape(128, 64, C).astype(BF16)),
                "a": np.ascontiguousarray(al.reshape(128, 64, K).astype(BF16)),
            }
        )
    kw1 = {}
    if trace:
        import tempfile
        d = tempfile.mkdtemp(prefix="p1_trace_")
        TRACE_DIRS.append(d)
        kw1 = dict(trace=True, tmpdir=d)
    res1 = run_bass_kernel_spmd(nc1, in_maps1, list(range(NCORES)), **kw1)
    exec1 = getattr(res1, "exec_time_ns", None)
    xp_parts = np.stack([res1.results[c]["xp"] for c in range(NCORES)])  # (8,64,256)

    # ---------------- host glue ----------------
    denom = A_all.sum(axis=1) + EPS  # (B, K)

    Wq3 = W_q.reshape(C, H, D)
    Wkcl_all, v3_all = [], []
    for b in range(B):
        xp = xp_parts[2 * b] + xp_parts[2 * b + 1]
        pooled = xp / denom[b][:, None]
        k_cl = pooled @ W_k
        v_cl = pooled @ W_v
        k3 = k_cl.reshape(K, H, D)
        Wkcl = np.einsum("chd,khd->chk", Wq3, k3).reshape(C, H * K) * SCALE
        Wkcl_all.append(Wkcl)
        v3_all.append(v_cl.reshape(K, H, D))

    Sk = _pow2_scale(max(np.abs(w).max() for w in Wkcl_all))
    Sv = _pow2_scale(max(np.abs(v).max() for v in v3_all))
    _GLOBAL_SCALES[0] = 1.0 / Sk
    _GLOBAL_SCALES[1] = Sv

    # constants shared by all cores
    cb2 = np.zeros((K, 128), np.float32)
    cb2[:, 0:64] = cb * Sk
    cb2[:, 64:128] = cb * Sk
    cb2_bf = cb2.astype(BF16)

    # wide denominator indicator: s lands pre-broadcast in attn-out layout.
    # sind2[p, j, m] = 1 iff 2*j + p//64 == m//32 (same for both P tiles).
    sind2 = np.zeros((128, 2, 128), np.float32)
    for p in range(128):
        for j in range(2):
            blk = 2 * j + p // 64
            sind2[p, j, 32 * blk : 32 * blk + 32] = 1.0
    sind2_8 = sind2.astype(E4)

    wpj = np.ascontiguousarray(
        w_proj_np.reshape(2, 128, C).transpose(1, 0, 2)
    ).astype(BF16)  # [p, ch, c]

    wkcl8_all, vbd8_all = [], []
    for b in range(B):
        # [p, m, ch, mcol]: chunk-m weight blocks with the 2 c-halves adjacent
        wk = (Wkcl_all[b] * Sk).reshape(2, 128, 4, 128).transpose(1, 2, 0, 3)
        wkcl8_all.append(np.ascontiguousarray(wk).astype(E4))
        v3s = v3_all[b] * Sv
        vbd = np.zeros((128, 2, 2, 128), np.float32)
        for t in range(2):
            for jj in range(2):
                for h2 in range(2):
                    c0 = jj * 64 + 32 * h2
                    vbd[64 * h2 : 64 * (h2 + 1), t, jj, c0 : c0 + 32] = v3s[
                        :, 4 * t + 2 * jj + h2, :
                    ]
        vbd8_all.append(vbd.astype(E4))

    # ---------------- pass 2 ----------------
    nc2 = _build_pass2(float(1.0 / Sk))
    in_maps2 = []
    for core in range(NCORES):
        b, half = core // 2, core % 2
        xl = x_all[b, half * NLOC : (half + 1) * NLOC]
        al = A_all[b, half * NLOC : (half + 1) * NLOC]
        in_maps2.append(
            {
                # [mg, p, gi, ch, nf]: per 512-col group, the 2 c-half
                # k-tiles adjacent in the free dim (DoubleRow layout)
                "xs": np.ascontiguousarray(
                    xl.T.reshape(2, 128, NMG, MG // F, F).transpose(2, 1, 3, 0, 4)
                ).astype(E4),
                "as_": np.ascontiguousarray(al.T).astype(BF16),
                "wkcl": wkcl8_all[b],
                "cb2": cb2_bf,
                "vbd": vbd8_all[b],
                "sind2": sind2_8,
                "wpj": wpj,
            }
        )
    kw2 = {}
    if trace:
        import tempfile
        d = tempfile.mkdtemp(prefix="p2_trace_")
        TRACE_DIRS.append(d)
        kw2 = dict(trace=True, tmpdir=d)
    res2 = run_bass_kernel_spmd(nc2, in_maps2, list(range(NCORES)), **kw2)
    exec2 = getattr(res2, "exec_time_ns", None)
    if exec1 is not None and exec2 is not None:
        LAST_EXEC_NS = exec1 + exec2
        globals()["LAST_EXEC_SPLIT"] = (exec1, exec2)

    inv_sv = 1.0 / Sv
    y_out = np.zeros((B, N, C), np.float32)
    for core in range(NCORES):
        b, half = core // 2, core % 2
        yv = res2.results[core]["y"].astype(np.float32)  # (2,128,NLOC), *Sv
        y_out[b, half * NLOC : (half + 1) * NLOC] = (
            yv.transpose(2, 0, 1).reshape(NLOC, C) * inv_sv
        )
    y_out += b_proj_np[None, None, :]
    return y_out

